# revision 1
# baseline (speedup 1.0000x reference)
"""Trainium2 Bass kernel for nn_AttnAware (pixnorm->conv1x1 q/k attention + ResnetBlock).

Sharding: 8 cores = 4 batches x 2 query-halves. Each core receives its batch's
x [256, 4096] with pixel columns rotated so that its 2048 query pixels are the
first 2048 columns (attention is permutation-invariant over keys, and all
other ops are per-pixel). Single SPMD program, no collectives.

Per-core data layout: channels on partitions, pixels on free axis.
Attention works in the S^T orientation: S^T[j,i] tiles [128 keys, i-chunk]
computed as k_block^T @ q (both naturally [head_dim, n]), exp on ACT (with the
1/sqrt(HD) scale fused), then O^T accumulated as V^T_block^T @ P^T with V^T
pre-transposed once per head on the PE. The softmax denominator (a
partition-axis sum) is computed by ones-row matmuls on the PE for some
j-groups and by DVE accumulate + a final ones-matmul fold for the rest
(D_PE_GROUPS knob balances PE vs DVE load). All big matmuls use float32r
(1 cycle/row, ~FP22 multiply precision, fp32 accumulate).
"""

import math
from contextlib import ExitStack

import numpy as np

import concourse.bass as bass
import concourse.mybir as mybir
import concourse.tile as tile
from concourse import bacc
from concourse.masks import make_identity

# ---------------- problem constants (hardcoded per contract) ----------------
B = 4
C = 256
HW = 64
N = HW * HW              # 4096 pixels
NQ = N // 2              # 2048 query pixels per core
NH = 2
HD = C // NH             # 128
CT = C // 128            # 2 channel tiles
C2T = 2 * C // 128       # 4 channel tiles for cat
JB = N // 128            # 32 key blocks
ATT_SCALE = HD ** -0.5
RATIO = 1.0 / (1.0 + 1e-8)   # PartialConv mask ratio (== 1.0f in fp32)
EPS = 1e-8
ISQ2 = 1.0 / math.sqrt(2.0)

# ---------------- tuning knobs ----------------
IW = 1024                # i-columns per attention pass (PSUM S tile width)
D_PE_JBS = 0            # j-blocks whose denominator goes via PE ones-matmul
                         # (the rest accumulate on DVE)
LDW_OPT = True           # enable walrus LDWEIGHTS dedupe/overlap optimization

f32 = mybir.dt.float32
f32r = mybir.dt.float32r
AF = mybir.ActivationFunctionType
OP = mybir.AluOpType


def r(ap):
    return ap.bitcast(f32r)


def build_program():
    nc = bacc.Bacc("TRN2", target_bir_lowering=False, debug=False)

    # register the pixnorm epsilon as a const AP usable as an ACT bias
    _eps_t = nc.alloc_sbuf_tensor(f"const-float32-{EPS}", [128, 1], f32)
    nc.gpsimd.memset(_eps_t.ap(), EPS)
    nc.const_aps.aps[(f32, EPS)] = _eps_t.ap()
    nc.all_engine_barrier()

    d = {}
    d["x"] = nc.dram_tensor("x", (C, N), f32, kind="ExternalInput").ap()
    d["wqT"] = nc.dram_tensor("wqT", (C, C), f32, kind="ExternalInput").ap()
    d["wkT"] = nc.dram_tensor("wkT", (C, C), f32, kind="ExternalInput").ap()
    d["wsT"] = nc.dram_tensor("wsT", (2 * C, C), f32, kind="ExternalInput").ap()
    d["w1T"] = nc.dram_tensor("w1T", (2 * C, C), f32, kind="ExternalInput").ap()
    d["w2T"] = nc.dram_tensor("w2T", (C, C), f32, kind="ExternalInput").ap()
    d["bq"] = nc.dram_tensor("bq", (C, 1), f32, kind="ExternalInput").ap()
    d["bk"] = nc.dram_tensor("bk", (C, 1), f32, kind="ExternalInput").ap()
    d["b1"] = nc.dram_tensor("b1", (C, 1), f32, kind="ExternalInput").ap()
    d["bsc"] = nc.dram_tensor("bsc", (C, 1), f32, kind="ExternalInput").ap()
    d["aq"] = nc.dram_tensor("aq", (C, 1), f32, kind="ExternalInput").ap()
    d["ak"] = nc.dram_tensor("ak", (C, 1), f32, kind="ExternalInput").ap()
    d["ar1"] = nc.dram_tensor("ar1", (2 * C, 1), f32, kind="ExternalInput").ap()
    d["ar2"] = nc.dram_tensor("ar2", (C, 1), f32, kind="ExternalInput").ap()
    d["y"] = nc.dram_tensor("y", (C, NQ), f32, kind="ExternalOutput").ap()

    with tile.TileContext(nc) as tc:
        _body(tc, nc, d)
    nc.compile()
    return nc


def _body(tc, nc, d):
    x_d, y_d = d["x"], d["y"]

    with ExitStack() as top:
        const = top.enter_context(tc.tile_pool(name="const", bufs=1))
        wts = top.enter_context(tc.tile_pool(name="wts", bufs=1))

        ident = const.tile([128, 128], f32, tag="ident", name="ident")
        make_identity(nc, ident[:])
        ones_col0 = const.tile([128, 1], f32, tag="ones_col0", name="ones_col0")
        nc.vector.memset(ones_col0[:], 1.0)
        ones_row0 = const.tile([1, 128], f32, tag="ones_row0", name="ones_row0")
        nc.vector.memset(ones_row0[:], 1.0)
        ones_col = const.tile([128, 1], f32, tag="ones_col", name="ones_col")
        nc.vector.tensor_copy(ones_col[:].bitcast(f32r), ones_col0[:])
        ones_row = const.tile([1, 128], f32, tag="ones_row", name="ones_row")
        nc.vector.tensor_copy(ones_row[:].bitcast(f32r), ones_row0[:])

        def load_split(name, n_tiles, width, rounded=False):
            ts = []
            for i in range(n_tiles):
                t = wts.tile([128, width], f32, tag=f"{name}{i}", name=f"{name}{i}")
                if rounded:
                    nc.sync.dma_start(t[:].bitcast(f32r),
                                      d[name][i * 128:(i + 1) * 128, :].bitcast(f32r))
                else:
                    nc.sync.dma_start(t[:], d[name][i * 128:(i + 1) * 128, :])
                ts.append(t)
            return ts

        wqT = load_split("wqT", CT, C, rounded=True)
        wkT = load_split("wkT", CT, C, rounded=True)
        wsT = load_split("wsT", C2T, C, rounded=True)
        w1T = load_split("w1T", C2T, C, rounded=True)
        w2T = load_split("w2T", CT, C, rounded=True)
        bq = load_split("bq", CT, 1)
        bk = load_split("bk", CT, 1)
        b1 = load_split("b1", CT, 1)
        bsc = load_split("bsc", CT, 1)
        aq = load_split("aq", CT, 1)
        ak = load_split("ak", CT, 1)
        ar1 = load_split("ar1", C2T, 1)
        ar2 = load_split("ar2", CT, 1)

        # oout: attention outputs, live into phase C
        with tc.tile_pool(name="oout", bufs=1) as oout:
            osb = [oout.tile([128, NQ], f32, tag=f"o{h}", name=f"o{h}") for h in range(NH)]

            # kqv: tensors that live from phase A through attention; closed
            # explicitly before the ResnetBlock pools open to reuse SBUF
            kqv_stack = ExitStack()
            kqv = kqv_stack.enter_context(tc.tile_pool(name="kqv", bufs=1))
            vt = [kqv.tile([128, N], f32, tag=f"vt{h}", name=f"vt{h}") for h in range(NH)]
            kt = [kqv.tile([128, N], f32, tag=f"k{h}", name=f"k{h}") for h in range(NH)]
            qt = [kqv.tile([128, NQ], f32, tag=f"q{h}", name=f"q{h}") for h in range(NH)]
            dinv = [kqv.tile([1, NQ], f32, tag=f"dinv{h}", name=f"dinv{h}") for h in range(NH)]

            # =========== Phase A ===========
            with (
                tc.tile_pool(name="front", bufs=1) as front,
                tc.tile_pool(name="gtmp", bufs=6) as gtmp,
                tc.tile_pool(name="frow", bufs=2) as frow,
                tc.tile_pool(name="psA", bufs=2, space="PSUM") as psA,
                tc.tile_pool(name="psAbc", bufs=1, space="PSUM") as psAbc,
                tc.tile_pool(name="psArow", bufs=2, space="PSUM") as psArow,
            ):
                xt = []
                for ct in range(CT):
                    t = front.tile([128, N], f32, tag=f"x{ct}", name=f"x{ct}")
                    nc.sync.dma_start(t[:], x_d[ct * 128:(ct + 1) * 128, :])
                    xt.append(t)

                # V^T per head: PE transpose, 4 blocks per PSUM bank
                for h in range(NH):
                    for qb in range(JB // 4):
                        tp = psA.tile([128, 512], f32, tag="scratch", name="scratch")
                        for rr in range(4):
                            jb = qb * 4 + rr
                            nc.tensor.transpose(
                                tp[:, rr * 128:(rr + 1) * 128],
                                xt[h][:, jb * 128:(jb + 1) * 128], ident[:])
                        nc.vector.tensor_copy(vt[h][:, qb * 512:(qb + 1) * 512].bitcast(f32r), tp[:])

                # pixelnorm stats: ssum_c x^2 -> inv = exp(-0.5*ln(ssum/C+eps)),
                # computed per 512-column chunk; inv chunks feed the K=1
                # broadcast matmuls for each pixel half
                def inv_chunk(cc):
                    sqc = []
                    for ct in range(CT):
                        t = gtmp.tile([128, 512], f32, tag="g", name="sqch")
                        nc.gpsimd.tensor_tensor(
                            t[:].bitcast(f32r), xt[ct][:, cc * 512:(cc + 1) * 512],
                            xt[ct][:, cc * 512:(cc + 1) * 512], op=OP.mult)
                        sqc.append(t)
                    ss = psArow.tile([1, 512], f32, tag="ssum", name="ssum")
                    for ct in range(CT):
                        nc.tensor.matmul(ss[:], r(ones_col[:]), r(sqc[ct][:]),
                                         start=(ct == 0), stop=(ct == CT - 1))
                    lt = frow.tile([1, 512], f32, tag="lnt", name="lnt")
                    nc.scalar.activation(lt[:], ss[:], AF.Ln, bias=EPS, scale=1.0 / C)
                    iv = frow.tile([1, 512], f32, tag="inv", name="inv", bufs=8)
                    nc.scalar.activation(iv[:].bitcast(f32r), lt[:], AF.Exp, scale=-0.5)
                    return iv

                # batch all pixelnorm stats first (single lnexp table residency)
                all_inv = [inv_chunk(cc) for cc in range(N // 512)]

                # broadcast of inv for one pixel half, as a 4-bank PSUM tile
                def half_bcast(half):
                    bc = psAbc.tile([128, NQ], f32, tag="bigbc", name="bigbc")
                    for cc in range(NQ // 512):
                        iv = all_inv[half * (NQ // 512) + cc]
                        nc.tensor.matmul(bc[:, cc * 512:(cc + 1) * 512],
                                         r(ones_row[:]), r(iv[:]),
                                         start=True, stop=True)
                    return bc

                # conv helper: stream xb=x*inv chunks through gelu into matmuls
                def conv_chunk(bc, half, cc, wT, alpha, bias, out_tiles):
                    gchunks = []
                    asl = slice(half * NQ + cc * 512, half * NQ + (cc + 1) * 512)
                    bsl = slice(cc * 512, (cc + 1) * 512)
                    for ct in range(CT):
                        g = gtmp.tile([128, 512], f32, tag="g", name="g")
                        nc.vector.tensor_tensor(g[:].bitcast(f32r), xt[ct][:, asl],
                                                bc[:, bsl], op=OP.mult)
                        nc.scalar.activation(g[:].bitcast(f32r), g[:], AF.Gelu, scale=alpha[ct][:])
                        gchunks.append(g)
                    for mo in range(CT):
                        ps = psA.tile([128, 512], f32, tag="scratch", name="scratch")
                        for kc in range(CT):
                            nc.tensor.matmul(ps[:],
                                             r(wT[kc][:, mo * 128:(mo + 1) * 128]),
                                             r(gchunks[kc][:]),
                                             start=(kc == 0), stop=(kc == CT - 1))
                        nc.vector.tensor_scalar(out_tiles[mo][:, asl].bitcast(f32r),
                                                ps[:], bias[mo][:], None, op0=OP.add)

                bc0 = half_bcast(0)
                for cc in range(NQ // 512):
                    conv_chunk(bc0, 0, cc, wqT, aq, bq, qt)
                for cc in range(NQ // 512):
                    conv_chunk(bc0, 0, cc, wkT, ak, bk, kt)
                bc1 = half_bcast(1)
                for cc in range(NQ // 512):
                    conv_chunk(bc1, 1, cc, wkT, ak, bk, kt)

            # =========== Phase B: attention (jb-outer; stationary weights
            # amortized across the whole 1024-wide i pass) ===========
            if True:
                with (
                    tc.tile_pool(name="psS", bufs=3, space="PSUM") as psS,
                    tc.tile_pool(name="psO", bufs=1, space="PSUM") as psO,
                    tc.tile_pool(name="pexp", bufs=3) as pexp,
                    tc.tile_pool(name="dacc", bufs=2) as dacc_pool,
                    tc.tile_pool(name="drow", bufs=2) as drow_pool,
                ):
                    NR = IW // 512
                    for h in range(NH):
                        for ip in range(NQ // IW):
                            i0 = ip * IW
                            o_ps = psO.tile([128, IW], f32, tag="o", name="o")
                            n_dve_jbs = JB - D_PE_JBS
                            dac = (dacc_pool.tile([128, IW], f32, tag="dacc",
                                                  name="dacc")
                                   if n_dve_jbs > 0 else None)
                            n_dve = 0
                            for jb in range(JB):
                                s_ps = psS.tile([128, IW], f32, tag="s", name="s")
                                for rr in range(NR):
                                    nc.tensor.matmul(
                                        s_ps[:, rr * 512:(rr + 1) * 512],
                                        r(kt[h][:, jb * 128:(jb + 1) * 128]),
                                        r(qt[h][:, i0 + rr * 512:i0 + (rr + 1) * 512]),
                                        start=True, stop=True)
                                p_sb = pexp.tile([128, IW], f32, tag="p", name="p")
                                nc.scalar.activation(p_sb[:].bitcast(f32r), s_ps[:],
                                                     AF.Exp, scale=ATT_SCALE)
                                for rr in range(NR):
                                    nc.tensor.matmul(
                                        o_ps[:, rr * 512:(rr + 1) * 512],
                                        r(vt[h][:, jb * 128:(jb + 1) * 128]),
                                        r(p_sb[:, rr * 512:(rr + 1) * 512]),
                                        start=(jb == 0), stop=(jb == JB - 1))
                                if jb < D_PE_JBS:
                                    for rr in range(NR):
                                        nc.tensor.matmul(
                                            d_ps[:, rr * 512:(rr + 1) * 512],
                                            r(ones_col[:]),
                                            r(p_sb[:, rr * 512:(rr + 1) * 512]),
                                            start=(jb == 0),
                                            stop=(jb == JB - 1 and n_dve_jbs == 0))
                                else:
                                    if n_dve == 0:
                                        nc.vector.tensor_copy(dac[:], p_sb[:])
                                    else:
                                        nc.vector.tensor_tensor(dac[:], dac[:],
                                                                p_sb[:], op=OP.add)
                                    n_dve += 1
                            if n_dve:
                                d_ps = psS.tile([1, IW], f32, tag="s", name="d")
                                dac_r = dacc_pool.tile([128, IW], f32, tag="daccr",
                                                       name="daccr")
                                nc.vector.tensor_copy(dac_r[:].bitcast(f32r), dac[:])
                                for rr in range(NR):
                                    nc.tensor.matmul(
                                        d_ps[:, rr * 512:(rr + 1) * 512],
                                        r(ones_col[:]),
                                        r(dac_r[:, rr * 512:(rr + 1) * 512]),
                                        start=(D_PE_JBS == 0), stop=True)
                            # Dinv = exp(-ln(D)) on ACT (lnexp set already live)
                            lrow = drow_pool.tile([1, IW], f32, tag="lrow",
                                                  name="lrow")
                            nc.scalar.activation(lrow[:], d_ps[:], AF.Ln)
                            nc.scalar.activation(
                                dinv[h][:, i0:i0 + IW].bitcast(f32r), lrow[:],
                                AF.Exp, scale=-1.0)
                            nc.vector.tensor_copy(
                                osb[h][:, i0:i0 + IW].bitcast(f32r), o_ps[:])

                # ======= Phase C: normalize O, ResnetBlock =======
                with (
                    tc.tile_pool(name="psBC", bufs=1, space="PSUM") as psBC,
                    tc.tile_pool(name="psB", bufs=2, space="PSUM") as psB,
                    tc.tile_pool(name="psBrow", bufs=2, space="PSUM") as psBrow,
                ):
                    def bcast_row(row_ap):
                        bc = psBC.tile([128, NQ], f32, tag="bigbc", name="bigbc")
                        for cc in range(NQ // 512):
                            nc.tensor.matmul(bc[:, cc * 512:(cc + 1) * 512],
                                             r(ones_row[:]),
                                             r(row_ap[:, cc * 512:(cc + 1) * 512]),
                                             start=True, stop=True)
                        return bc

                    # O /= D
                    for h in range(NH):
                        bc = bcast_row(dinv[h][:])
                        nc.vector.tensor_tensor(osb[h][:].bitcast(f32r), osb[h][:],
                                                bc[:], op=OP.mult)
                # kqv pool (k/q/vt/dinv) closes here; back pool reuses its space
                kqv_stack.close()
                with (
                    tc.tile_pool(name="back", bufs=1) as back,
                    tc.tile_pool(name="brow", bufs=4) as brow,
                    tc.tile_pool(name="tmp", bufs=4) as tmp,
                    tc.tile_pool(name="psBC2", bufs=1, space="PSUM") as psBC2,
                    tc.tile_pool(name="psB2", bufs=2, space="PSUM") as psB2,
                    tc.tile_pool(name="psBrow2", bufs=2, space="PSUM") as psBrow2,
                ):
                    xq = []
                    for ct in range(CT):
                        t = back.tile([128, NQ], f32, tag=f"xq{ct}", name=f"xq{ct}")
                        nc.sync.dma_start(t[:].bitcast(f32r),
                                          x_d[ct * 128:(ct + 1) * 128, :NQ].bitcast(f32r))
                        xq.append(t)
                    cat = [osb[0], osb[1], xq[0], xq[1]]

                    def stats(tiles, nch, tag):
                        out_chunks = []
                        for cc in range(NQ // 512):
                            ss = psBrow2.tile([1, 512], f32, tag="ssum", name="ssum")
                            for i, t in enumerate(tiles):
                                nc.tensor.matmul(ss[:], r(ones_col[:]),
                                                 r(t[:, cc * 512:(cc + 1) * 512]),
                                                 start=(i == 0),
                                                 stop=(i == len(tiles) - 1))
                            lt = brow.tile([1, 512], f32, tag="lnt", name="lnt")
                            nc.scalar.activation(lt[:], ss[:], AF.Ln, bias=EPS,
                                                 scale=1.0 / nch)
                            iv = brow.tile([1, 512], f32, tag=f"iv{tag}", name=f"iv{tag}")
                            nc.scalar.activation(iv[:].bitcast(f32r), lt[:], AF.Exp,
                                                 scale=-0.5)
                            out_chunks.append(iv)
                        return out_chunks

                    def bcast_chunks(chunks):
                        bc = psBC2.tile([128, NQ], f32, tag="bigbc", name="bigbc")
                        for cc in range(NQ // 512):
                            nc.tensor.matmul(bc[:, cc * 512:(cc + 1) * 512],
                                             r(ones_row[:]), r(chunks[cc][:]),
                                             start=True, stop=True)
                        return bc

                    # r1 stats over 512 channels of cat
                    sqc = []
                    for ct in range(C2T):
                        t = tmp.tile([128, NQ], f32, tag="sqc", name="sqc")
                        nc.gpsimd.tensor_tensor(t[:].bitcast(f32r), cat[ct][:],
                                                cat[ct][:], op=OP.mult)
                        sqc.append(t)
                    invr1 = stats(sqc, 2 * C, "r1")

                    # x_short (scaled by 1/sqrt2; bias (bs+b2)/sqrt2)
                    xs = [back.tile([128, NQ], f32, tag=f"xs{mo}", name=f"xs{mo}") for mo in range(CT)]
                    for mo in range(CT):
                        for cc in range(NQ // 512):
                            ps = psB2.tile([128, 512], f32, tag="conv", name="conv")
                            for kc in range(C2T):
                                nc.tensor.matmul(
                                    ps[:], r(wsT[kc][:, mo * 128:(mo + 1) * 128]),
                                    r(cat[kc][:, cc * 512:(cc + 1) * 512]),
                                    start=(kc == 0), stop=(kc == C2T - 1))
                            nc.vector.tensor_scalar(
                                xs[mo][:, cc * 512:(cc + 1) * 512], ps[:],
                                RATIO * ISQ2, bsc[mo][:], op0=OP.mult, op1=OP.add)

                    # gr1 = gelu(alpha_r1 * cat * invr1)
                    bc1 = bcast_chunks(invr1)
                    gr1 = []
                    for ct in range(C2T):
                        cn = tmp.tile([128, NQ], f32, tag="sqc", name="sqc")
                        nc.vector.tensor_tensor(cn[:], cat[ct][:], bc1[:], op=OP.mult)
                        t = back.tile([128, NQ], f32, tag=f"gr1{ct}", name=f"gr1{ct}")
                        nc.scalar.activation(t[:].bitcast(f32r), cn[:], AF.Gelu,
                                             scale=ar1[ct][:])
                        gr1.append(t)

                    # h1 = W1 @ gr1 * ratio + b1
                    h1 = [back.tile([128, NQ], f32, tag=f"h1{mo}", name=f"h1{mo}") for mo in range(CT)]
                    for mo in range(CT):
                        for cc in range(NQ // 512):
                            ps = psB2.tile([128, 512], f32, tag="conv", name="conv")
                            for kc in range(C2T):
                                nc.tensor.matmul(
                                    ps[:], r(w1T[kc][:, mo * 128:(mo + 1) * 128]),
                                    r(gr1[kc][:, cc * 512:(cc + 1) * 512]),
                                    start=(kc == 0), stop=(kc == C2T - 1))
                            nc.vector.tensor_scalar(
                                h1[mo][:, cc * 512:(cc + 1) * 512], ps[:],
                                RATIO, b1[mo][:], op0=OP.mult, op1=OP.add)

                    # r2 stats over h1
                    sqh = []
                    for ct in range(CT):
                        t = tmp.tile([128, NQ], f32, tag="sqc", name="sqc")
                        nc.gpsimd.tensor_tensor(t[:].bitcast(f32r), h1[ct][:], h1[ct][:], op=OP.mult)
                        sqh.append(t)
                    invr2 = stats(sqh, C, "r2")

                    # gr2 = gelu(alpha_r2 * h1 * invr2)  (h1 scaled in place)
                    bc2 = bcast_chunks(invr2)
                    gr2 = []
                    for ct in range(CT):
                        nc.vector.tensor_tensor(h1[ct][:], h1[ct][:], bc2[:],
                                                op=OP.mult)
                        t = back.tile([128, NQ], f32, tag=f"gr1{ct}", name=f"gr1{ct}")
                        nc.scalar.activation(t[:].bitcast(f32r), h1[ct][:], AF.Gelu,
                                             scale=ar2[ct][:])
                        gr2.append(t)

                    # y = W2 @ gr2 * ratio/sqrt2 + xs
                    for mo in range(CT):
                        yt = back.tile([128, NQ], f32, tag=f"gr1{mo + 2}", name=f"gr1{mo + 2}")
                        for cc in range(NQ // 512):
                            ps = psB2.tile([128, 512], f32, tag="conv", name="conv")
                            for kc in range(CT):
                                nc.tensor.matmul(
                                    ps[:], r(w2T[kc][:, mo * 128:(mo + 1) * 128]),
                                    r(gr2[kc][:, cc * 512:(cc + 1) * 512]),
                                    start=(kc == 0), stop=(kc == CT - 1))
                            nc.vector.scalar_tensor_tensor(
                                yt[:, cc * 512:(cc + 1) * 512], ps[:], RATIO * ISQ2,
                                xs[mo][:, cc * 512:(cc + 1) * 512],
                                op0=OP.mult, op1=OP.add)
                        nc.sync.dma_start(y_d[mo * 128:(mo + 1) * 128, :], yt[:])


_PROGRAM = None


def get_program():
    global _PROGRAM
    if _PROGRAM is None:
        _PROGRAM = build_program()
    return _PROGRAM


def make_in_maps(inputs):
    x = np.asarray(inputs["x"], np.float32).reshape(B, C, N)
    col = lambda v, n: np.ascontiguousarray(np.asarray(v, np.float32).reshape(n, 1))
    tr = lambda w: np.ascontiguousarray(np.asarray(w, np.float32).T)
    shared = {
        "wqT": tr(inputs["Wq"]), "wkT": tr(inputs["Wk"]), "wsT": tr(inputs["Ws"]),
        "w1T": tr(inputs["W1"]), "w2T": tr(inputs["W2"]),
        "bq": col(inputs["bq"], C), "bk": col(inputs["bk"], C),
        "b1": col(inputs["b1"], C),
        "bsc": ((col(inputs["bs"], C).astype(np.float64) +
                 col(inputs["b2"], C).astype(np.float64)) * ISQ2).astype(np.float32),
        "aq": col(inputs["alpha_q"], C), "ak": col(inputs["alpha_k"], C),
        "ar1": col(inputs["alpha_r1"], 2 * C), "ar2": col(inputs["alpha_r2"], C),
    }
    in_maps = []
    for b in range(B):
        for half in range(2):
            xp = (np.ascontiguousarray(x[b]) if half == 0
                  else np.ascontiguousarray(np.roll(x[b], -NQ, axis=1)))
            in_maps.append({"x": xp, **shared})
    return in_maps


def assemble_output(results):
    y = np.empty((B, C, N), np.float32)
    for core, res in enumerate(results):
        b, half = core // 2, core % 2
        y[b][:, half * NQ:(half + 1) * NQ] = res["y"]
    return y.reshape(B, C, HW, HW)


def _patch_ldw_opt():
    from concourse import bass_utils
    if getattr(bass_utils, "_ldw_patched", False):
        return
    orig = bass_utils.run_command

    def patched(argv, **kw):
        argv = ["--enable-ldw-opt=true" if a == "--enable-ldw-opt=false" else a
                for a in argv]
        return orig(argv, **kw)

    bass_utils.run_command = patched
    bass_utils._ldw_patched = True


def kernel(**inputs):
    from concourse.bass_utils import run_bass_kernel_spmd

    if LDW_OPT:
        _patch_ldw_opt()
    nc = get_program()
    in_maps = make_in_maps(inputs)
    out = run_bass_kernel_spmd(nc, in_maps, core_ids=list(range(8)))
    return assemble_output(out.results)


if __name__ == "__main__":
    get_program()
    print("built ok")



# revision 6
# speedup vs baseline: 1.0549x; 1.0549x over previous
"""Trainium2 Bass kernel for nn_AttnAware (pixnorm->conv1x1 q/k attention + ResnetBlock).

Sharding: 8 cores = 4 batches x 2 query-halves. Each core receives its batch's
x [256, 4096] with pixel columns rotated so that its 2048 query pixels are the
first 2048 columns (attention is permutation-invariant over keys, and all
other ops are per-pixel). Single SPMD program, no collectives.

Per-core data layout: channels on partitions, pixels on free axis.
Attention works in the S^T orientation: S^T[j,i] tiles [128 keys, i-chunk]
computed as k_block^T @ q (both f32r), exp on ACT (scale fused) writing fp16
P tiles, O^T accumulated as V^T_block^T @ P^T in fp16 (V^T pre-transposed
once per head on the PE from an fp16 copy of x). The softmax denominator
accumulates fp16 P tiles on DVE (2x rate), folds to a [1,IW] row via a
ones-column matmul, takes 1/D with the native DVE reciprocal, broadcasts via
a ones-row matmul and normalizes O inline per (head, i-pass).
"""

import math
from contextlib import ExitStack

import numpy as np

import concourse.bass as bass
import concourse.mybir as mybir
import concourse.tile as tile
from concourse import bacc
from concourse.masks import make_identity

# ---------------- problem constants (hardcoded per contract) ----------------
B = 4
C = 256
HW = 64
N = HW * HW              # 4096 pixels
NQ = N // 2              # 2048 query pixels per core
NH = 2
HD = C // NH             # 128
CT = C // 128            # 2 channel tiles
C2T = 2 * C // 128       # 4 channel tiles for cat
JB = N // 128            # 32 key blocks
ATT_SCALE = HD ** -0.5
RATIO = 1.0 / (1.0 + 1e-8)   # PartialConv mask ratio (== 1.0f in fp32)
EPS = 1e-8
ISQ2 = 1.0 / math.sqrt(2.0)

# ---------------- tuning knobs ----------------
IW = 1024                # i-columns per attention pass (PSUM S tile width)
LDW_OPT = False           # enable walrus LDWEIGHTS dedupe/overlap optimization

f32 = mybir.dt.float32
f32r = mybir.dt.float32r
f16 = mybir.dt.float16
AF = mybir.ActivationFunctionType
OP = mybir.AluOpType


def r(ap):
    return ap.bitcast(f32r)


def build_program():
    nc = bacc.Bacc("TRN2", target_bir_lowering=False, debug=False)

    # register the pixnorm epsilon as a const AP usable as an ACT bias
    _eps_t = nc.alloc_sbuf_tensor(f"const-float32-{EPS}", [128, 1], f32)
    nc.gpsimd.memset(_eps_t.ap(), EPS)
    nc.const_aps.aps[(f32, EPS)] = _eps_t.ap()
    nc.all_engine_barrier()

    d = {}
    d["x"] = nc.dram_tensor("x", (C, N), f32, kind="ExternalInput").ap()
    d["wqT"] = nc.dram_tensor("wqT", (C, C), f32, kind="ExternalInput").ap()
    d["wkT"] = nc.dram_tensor("wkT", (C, C), f32, kind="ExternalInput").ap()
    d["wsT"] = nc.dram_tensor("wsT", (2 * C, C), f32, kind="ExternalInput").ap()
    d["w1T"] = nc.dram_tensor("w1T", (2 * C, C), f32, kind="ExternalInput").ap()
    d["w2T"] = nc.dram_tensor("w2T", (C, C), f32, kind="ExternalInput").ap()
    d["bq"] = nc.dram_tensor("bq", (C, 1), f32, kind="ExternalInput").ap()
    d["bk"] = nc.dram_tensor("bk", (C, 1), f32, kind="ExternalInput").ap()
    d["b1"] = nc.dram_tensor("b1", (C, 1), f32, kind="ExternalInput").ap()
    d["bsc"] = nc.dram_tensor("bsc", (C, 1), f32, kind="ExternalInput").ap()
    d["aq"] = nc.dram_tensor("aq", (C, 1), f32, kind="ExternalInput").ap()
    d["ak"] = nc.dram_tensor("ak", (C, 1), f32, kind="ExternalInput").ap()
    d["ar1"] = nc.dram_tensor("ar1", (2 * C, 1), f32, kind="ExternalInput").ap()
    d["ar2"] = nc.dram_tensor("ar2", (C, 1), f32, kind="ExternalInput").ap()
    d["y"] = nc.dram_tensor("y", (C, NQ), f32, kind="ExternalOutput").ap()

    with tile.TileContext(nc) as tc:
        _body(tc, nc, d)
    nc.compile()
    return nc


def _body(tc, nc, d):
    x_d, y_d = d["x"], d["y"]

    with ExitStack() as top:
        const = top.enter_context(tc.tile_pool(name="const", bufs=1))
        wts = top.enter_context(tc.tile_pool(name="wts", bufs=1))
        # xt lives from phase A through phase C (cat reads x columns)
        front = top.enter_context(tc.tile_pool(name="front", bufs=1))

        ident16 = const.tile([128, 128], f16, tag="ident16", name="ident16")
        make_identity(nc, ident16[:])
        ones_col0 = const.tile([128, 1], f32, tag="ones_col0", name="ones_col0")
        nc.vector.memset(ones_col0[:], 1.0)
        ones_row0 = const.tile([1, 128], f32, tag="ones_row0", name="ones_row0")
        nc.vector.memset(ones_row0[:], 1.0)
        ones_col = const.tile([128, 1], f32, tag="ones_col", name="ones_col")
        nc.vector.tensor_copy(ones_col[:].bitcast(f32r), ones_col0[:])
        ones_row = const.tile([1, 128], f32, tag="ones_row", name="ones_row")
        nc.vector.tensor_copy(ones_row[:].bitcast(f32r), ones_row0[:])
        ones_col16 = const.tile([128, 1], f16, tag="ones_col16", name="ones_col16")
        nc.vector.memset(ones_col16[:], 1.0)

        # x DMA first (critical path), in 1024-col chunks so downstream
        # work starts as soon as the first chunks land
        xt = [front.tile([128, N], f32, tag=f"x{ct}", name=f"x{ct}")
              for ct in range(CT)]
        for q in range(N // 1024):
            for ct in range(CT):
                nc.sync.dma_start(
                    xt[ct][:, q * 1024:(q + 1) * 1024].bitcast(f32r),
                    x_d[ct * 128:(ct + 1) * 128,
                        q * 1024:(q + 1) * 1024].bitcast(f32r))

        def load_split(name, n_tiles, width, rounded=False):
            ts = []
            for i in range(n_tiles):
                t = wts.tile([128, width], f32, tag=f"{name}{i}", name=f"{name}{i}")
                if rounded:
                    nc.sync.dma_start(t[:].bitcast(f32r),
                                      d[name][i * 128:(i + 1) * 128, :].bitcast(f32r))
                else:
                    nc.sync.dma_start(t[:], d[name][i * 128:(i + 1) * 128, :])
                ts.append(t)
            return ts

        wqT = load_split("wqT", CT, C, rounded=True)
        wkT = load_split("wkT", CT, C, rounded=True)
        wsT = load_split("wsT", C2T, C, rounded=True)
        w1T = load_split("w1T", C2T, C, rounded=True)
        w2T = load_split("w2T", CT, C, rounded=True)
        bq = load_split("bq", CT, 1)
        bk = load_split("bk", CT, 1)
        b1 = load_split("b1", CT, 1)
        bsc = load_split("bsc", CT, 1)
        aq = load_split("aq", CT, 1)
        ak = load_split("ak", CT, 1)
        ar1 = load_split("ar1", C2T, 1)
        ar2 = load_split("ar2", CT, 1)

        # oout: attention outputs, live into phase C
        with tc.tile_pool(name="oout", bufs=1) as oout:
            osb = [oout.tile([128, NQ], f32, tag=f"o{h}", name=f"o{h}") for h in range(NH)]

            # kqv: tensors that live from phase A through attention; closed
            # explicitly before the ResnetBlock pools open to reuse SBUF
            kqv_stack = ExitStack()
            kqv = kqv_stack.enter_context(tc.tile_pool(name="kqv", bufs=1))
            x16 = [kqv.tile([128, N], f16, tag=f"x16_{h}", name=f"x16_{h}")
                   for h in range(NH)]
            vt = [kqv.tile([128, N], f16, tag=f"vt{h}", name=f"vt{h}") for h in range(NH)]
            kt = [kqv.tile([128, N], f32, tag=f"k{h}", name=f"k{h}") for h in range(NH)]
            qt = [kqv.tile([128, NQ], f32, tag=f"q{h}", name=f"q{h}") for h in range(NH)]

            # =========== Phase A ===========
            with (
                tc.tile_pool(name="gtmp", bufs=6) as gtmp,
                tc.tile_pool(name="frow", bufs=2) as frow,
                tc.tile_pool(name="psA", bufs=2, space="PSUM") as psA,
                tc.tile_pool(name="psArow", bufs=2, space="PSUM") as psArow,
            ):
                # fp16 copy of x (Pool engine), per head tile
                for q in range(N // 1024):
                    for h in range(NH):
                        nc.gpsimd.tensor_copy(
                            x16[h][:, q * 1024:(q + 1) * 1024],
                            xt[h][:, q * 1024:(q + 1) * 1024])

                # V^T per head: PE transpose of fp16 x, 4 blocks per PSUM slot
                for h in range(NH):
                    for qb in range(JB // 4):
                        tp = psA.tile([128, 512], f16, tag="tp", name="tp")
                        for rr in range(4):
                            jb = qb * 4 + rr
                            nc.tensor.transpose(
                                tp[:, rr * 128:(rr + 1) * 128],
                                x16[h][:, jb * 128:(jb + 1) * 128], ident16[:])
                        nc.vector.tensor_copy(vt[h][:, qb * 512:(qb + 1) * 512], tp[:])

                # pixelnorm stats: ssum_c x^2 -> inv = exp(-0.5*ln(ssum/C+eps)),
                # computed per 512-column chunk
                def inv_chunk(cc):
                    sqc = []
                    for ct in range(CT):
                        t = gtmp.tile([128, 512], f32, tag="g", name="sqch")
                        nc.gpsimd.tensor_tensor(
                            t[:].bitcast(f32r), xt[ct][:, cc * 512:(cc + 1) * 512],
                            xt[ct][:, cc * 512:(cc + 1) * 512], op=OP.mult)
                        sqc.append(t)
                    ss = psArow.tile([1, 512], f32, tag="ssum", name="ssum")
                    for ct in range(CT):
                        nc.tensor.matmul(ss[:], r(ones_col[:]), r(sqc[ct][:]),
                                         start=(ct == 0), stop=(ct == CT - 1))
                    lt = frow.tile([1, 512], f32, tag="lnt", name="lnt")
                    nc.scalar.activation(lt[:], ss[:], AF.Ln, bias=EPS, scale=1.0 / C)
                    iv = frow.tile([1, 512], f32, tag="inv", name="inv", bufs=8)
                    nc.scalar.activation(iv[:].bitcast(f32r), lt[:], AF.Exp, scale=-0.5)
                    return iv

                # batch all pixelnorm stats first (single lnexp table residency)
                all_inv = [inv_chunk(cc) for cc in range(N // 512)]

                # per-chunk: bcast inv, xb = x*inv, gelu per conv, matmuls
                def conv_block(cc, convs):
                    # convs: list of (wT, alpha, bias, out_tiles, out_col0)
                    bc = psA.tile([128, 512], f32, tag="bc", name="bc")
                    nc.tensor.matmul(bc[:], r(ones_row[:]), r(all_inv[cc][:]),
                                     start=True, stop=True)
                    xb = []
                    for ct in range(CT):
                        t = gtmp.tile([128, 512], f32, tag="g", name="xb")
                        nc.vector.tensor_tensor(
                            t[:].bitcast(f32r),
                            xt[ct][:, cc * 512:(cc + 1) * 512], bc[:], op=OP.mult)
                        xb.append(t)
                    for (wT, alpha, bias, out_tiles, oc0) in convs:
                        gchunks = []
                        for ct in range(CT):
                            g = gtmp.tile([128, 512], f32, tag="g", name="g")
                            nc.scalar.activation(g[:].bitcast(f32r), xb[ct][:],
                                                 AF.Gelu, scale=alpha[ct][:])
                            gchunks.append(g)
                        osl = slice(oc0 + (cc % 4) * 512, oc0 + (cc % 4 + 1) * 512)
                        for mo in range(CT):
                            ps = psA.tile([128, 512], f32, tag="conv", name="conv")
                            for kc in range(CT):
                                nc.tensor.matmul(ps[:],
                                                 r(wT[kc][:, mo * 128:(mo + 1) * 128]),
                                                 r(gchunks[kc][:]),
                                                 start=(kc == 0), stop=(kc == CT - 1))
                            nc.vector.tensor_scalar(out_tiles[mo][:, osl].bitcast(f32r),
                                                    ps[:], bias[mo][:], None, op0=OP.add)

                for cc in range(NQ // 512):
                    conv_block(cc, [(wqT, aq, bq, qt, 0), (wkT, ak, bk, kt, 0)])
                for cc in range(NQ // 512, N // 512):
                    conv_block(cc, [(wkT, ak, bk, kt, NQ)])

            # =========== Phase B: attention ===========
            with (
                tc.tile_pool(name="psS", bufs=3, space="PSUM") as psS,
                tc.tile_pool(name="psO", bufs=1, space="PSUM") as psO,
                tc.tile_pool(name="pexp", bufs=3) as pexp,
                tc.tile_pool(name="dacc", bufs=2) as dacc_pool,
                tc.tile_pool(name="drow", bufs=2) as drow_pool,
            ):
                NR = IW // 512
                for h in range(NH):
                    for ip in range(NQ // IW):
                        i0 = ip * IW
                        o_ps = psO.tile([128, IW], f32, tag="o", name="o")
                        dac = dacc_pool.tile([128, IW], f16, tag="dacc", name="dacc")
                        for jb in range(JB):
                            s_ps = psS.tile([128, IW], f32, tag="s", name="s")
                            for rr in range(NR):
                                nc.tensor.matmul(
                                    s_ps[:, rr * 512:(rr + 1) * 512],
                                    r(kt[h][:, jb * 128:(jb + 1) * 128]),
                                    r(qt[h][:, i0 + rr * 512:i0 + (rr + 1) * 512]),
                                    start=True, stop=True)
                            p_sb = pexp.tile([128, IW], f16, tag="p", name="p")
                            nc.scalar.activation(p_sb[:], s_ps[:],
                                                 AF.Exp, scale=ATT_SCALE)
                            for rr in range(NR):
                                nc.tensor.matmul(
                                    o_ps[:, rr * 512:(rr + 1) * 512],
                                    vt[h][:, jb * 128:(jb + 1) * 128],
                                    p_sb[:, rr * 512:(rr + 1) * 512],
                                    start=(jb == 0), stop=(jb == JB - 1))
                            if jb == 0:
                                nc.vector.tensor_copy(dac[:], p_sb[:])
                            else:
                                nc.vector.tensor_tensor(dac[:], dac[:],
                                                        p_sb[:], op=OP.add)
                        # fold dacc over partitions -> D row, then 1/D on DVE
                        d_ps = psS.tile([1, IW], f32, tag="s", name="d")
                        for rr in range(NR):
                            nc.tensor.matmul(
                                d_ps[:, rr * 512:(rr + 1) * 512],
                                ones_col16[:],
                                dac[:, rr * 512:(rr + 1) * 512],
                                start=True, stop=True)
                        dinv = drow_pool.tile([1, IW], f32, tag="dinv", name="dinv")
                        nc.vector.reciprocal(dinv[:], d_ps[:])
                        # bcast 1/D across partitions (Pool) and normalize O
                        bc = drow_pool.tile([128, IW], f32, tag="bcD", name="bcD")
                        nc.gpsimd.partition_broadcast(bc[:], dinv[:])
                        nc.vector.tensor_tensor(
                            osb[h][:, i0:i0 + IW].bitcast(f32r), o_ps[:], bc[:],
                            op=OP.mult)

            # kqv pool (k/q/vt/x16) closes here; back pool reuses its space
            kqv_stack.close()

            # ======= Phase C: ResnetBlock =======
            with (
                tc.tile_pool(name="back", bufs=1) as back,
                tc.tile_pool(name="brow", bufs=4) as brow,
                tc.tile_pool(name="tmp", bufs=4) as tmp,
                tc.tile_pool(name="psBC2", bufs=1, space="PSUM") as psBC2,
                tc.tile_pool(name="psB2", bufs=2, space="PSUM") as psB2,
                tc.tile_pool(name="psBrow2", bufs=2, space="PSUM") as psBrow2,
            ):
                cat = [osb[0], osb[1], xt[0], xt[1]]  # xt sliced to [:, :NQ]

                def cat_sl(t, cc):
                    return t[:, cc * 512:(cc + 1) * 512]

                def stats(tiles, nch, tag, width=NQ):
                    out_chunks = []
                    for cc in range(width // 512):
                        ss = psBrow2.tile([1, 512], f32, tag="ssum", name="ssum")
                        for i, t in enumerate(tiles):
                            nc.tensor.matmul(ss[:], r(ones_col[:]),
                                             r(cat_sl(t, cc)),
                                             start=(i == 0),
                                             stop=(i == len(tiles) - 1))
                        lt = brow.tile([1, 512], f32, tag="lnt", name="lnt")
                        nc.scalar.activation(lt[:], ss[:], AF.Ln, bias=EPS,
                                             scale=1.0 / nch)
                        iv = brow.tile([1, 512], f32, tag=f"iv{tag}", name=f"iv{tag}")
                        nc.scalar.activation(iv[:].bitcast(f32r), lt[:], AF.Exp,
                                             scale=-0.5)
                        out_chunks.append(iv)
                    return out_chunks

                def bcast_chunks(chunks):
                    bc = psBC2.tile([128, NQ], f32, tag="bigbc", name="bigbc")
                    for cc in range(NQ // 512):
                        nc.tensor.matmul(bc[:, cc * 512:(cc + 1) * 512],
                                         r(ones_row[:]), r(chunks[cc][:]),
                                         start=True, stop=True)
                    return bc

                # r1 stats over 512 channels of cat
                sqc = []
                for ct in range(C2T):
                    t = tmp.tile([128, NQ], f32, tag="sqc", name="sqc")
                    nc.gpsimd.tensor_tensor(t[:].bitcast(f32r), cat[ct][:, :NQ],
                                            cat[ct][:, :NQ], op=OP.mult)
                    sqc.append(t)
                invr1 = stats(sqc, 2 * C, "r1")

                # x_short (scaled by 1/sqrt2; bias (bs+b2)/sqrt2)
                xs = [back.tile([128, NQ], f32, tag=f"xs{mo}", name=f"xs{mo}") for mo in range(CT)]
                for mo in range(CT):
                    for cc in range(NQ // 512):
                        ps = psB2.tile([128, 512], f32, tag="conv", name="conv")
                        for kc in range(C2T):
                            nc.tensor.matmul(
                                ps[:], r(wsT[kc][:, mo * 128:(mo + 1) * 128]),
                                r(cat_sl(cat[kc], cc)),
                                start=(kc == 0), stop=(kc == C2T - 1))
                        nc.vector.tensor_scalar(
                            xs[mo][:, cc * 512:(cc + 1) * 512], ps[:],
                            RATIO * ISQ2, bsc[mo][:], op0=OP.mult, op1=OP.add)

                # gr1 = gelu(alpha_r1 * cat * invr1)
                bc1 = bcast_chunks(invr1)
                gr1 = []
                for ct in range(C2T):
                    cn = tmp.tile([128, NQ], f32, tag="sqc", name="sqc")
                    nc.vector.tensor_tensor(cn[:], cat[ct][:, :NQ], bc1[:], op=OP.mult)
                    t = back.tile([128, NQ], f32, tag=f"gr1{ct}", name=f"gr1{ct}")
                    nc.scalar.activation(t[:].bitcast(f32r), cn[:], AF.Gelu,
                                         scale=ar1[ct][:])
                    gr1.append(t)

                # h1 = W1 @ gr1 * ratio + b1
                h1 = [back.tile([128, NQ], f32, tag=f"h1{mo}", name=f"h1{mo}") for mo in range(CT)]
                for mo in range(CT):
                    for cc in range(NQ // 512):
                        ps = psB2.tile([128, 512], f32, tag="conv", name="conv")
                        for kc in range(C2T):
                            nc.tensor.matmul(
                                ps[:], r(w1T[kc][:, mo * 128:(mo + 1) * 128]),
                                r(gr1[kc][:, cc * 512:(cc + 1) * 512]),
                                start=(kc == 0), stop=(kc == C2T - 1))
                        nc.vector.tensor_scalar(
                            h1[mo][:, cc * 512:(cc + 1) * 512], ps[:],
                            RATIO, b1[mo][:], op0=OP.mult, op1=OP.add)

                # r2 stats over h1
                sqh = []
                for ct in range(CT):
                    t = tmp.tile([128, NQ], f32, tag="sqc", name="sqc")
                    nc.gpsimd.tensor_tensor(t[:].bitcast(f32r), h1[ct][:], h1[ct][:], op=OP.mult)
                    sqh.append(t)
                invr2 = stats(sqh, C, "r2")

                # gr2 = gelu(alpha_r2 * h1 * invr2)  (h1 scaled in place)
                bc2 = bcast_chunks(invr2)
                gr2 = []
                for ct in range(CT):
                    nc.vector.tensor_tensor(h1[ct][:], h1[ct][:], bc2[:],
                                            op=OP.mult)
                    t = back.tile([128, NQ], f32, tag=f"gr1{ct}", name=f"gr1{ct}")
                    nc.scalar.activation(t[:].bitcast(f32r), h1[ct][:], AF.Gelu,
                                         scale=ar2[ct][:])
                    gr2.append(t)

                # y = W2 @ gr2 * ratio/sqrt2 + xs
                for mo in range(CT):
                    yt = back.tile([128, NQ], f32, tag=f"gr1{mo + 2}", name=f"gr1{mo + 2}")
                    for cc in range(NQ // 512):
                        ps = psB2.tile([128, 512], f32, tag="conv", name="conv")
                        for kc in range(CT):
                            nc.tensor.matmul(
                                ps[:], r(w2T[kc][:, mo * 128:(mo + 1) * 128]),
                                r(gr2[kc][:, cc * 512:(cc + 1) * 512]),
                                start=(kc == 0), stop=(kc == CT - 1))
                        nc.vector.scalar_tensor_tensor(
                            yt[:, cc * 512:(cc + 1) * 512], ps[:], RATIO * ISQ2,
                            xs[mo][:, cc * 512:(cc + 1) * 512],
                            op0=OP.mult, op1=OP.add)
                    nc.sync.dma_start(y_d[mo * 128:(mo + 1) * 128, :], yt[:])


_PROGRAM = None


def get_program():
    global _PROGRAM
    if _PROGRAM is None:
        _PROGRAM = build_program()
    return _PROGRAM


def make_in_maps(inputs):
    x = np.asarray(inputs["x"], np.float32).reshape(B, C, N)
    col = lambda v, n: np.ascontiguousarray(np.asarray(v, np.float32).reshape(n, 1))
    tr = lambda w: np.ascontiguousarray(np.asarray(w, np.float32).T)
    shared = {
        "wqT": tr(inputs["Wq"]), "wkT": tr(inputs["Wk"]), "wsT": tr(inputs["Ws"]),
        "w1T": tr(inputs["W1"]), "w2T": tr(inputs["W2"]),
        "bq": col(inputs["bq"], C), "bk": col(inputs["bk"], C),
        "b1": col(inputs["b1"], C),
        "bsc": ((col(inputs["bs"], C).astype(np.float64) +
                 col(inputs["b2"], C).astype(np.float64)) * ISQ2).astype(np.float32),
        "aq": col(inputs["alpha_q"], C), "ak": col(inputs["alpha_k"], C),
        "ar1": col(inputs["alpha_r1"], 2 * C), "ar2": col(inputs["alpha_r2"], C),
    }
    in_maps = []
    for b in range(B):
        for half in range(2):
            xp = (np.ascontiguousarray(x[b]) if half == 0
                  else np.ascontiguousarray(np.roll(x[b], -NQ, axis=1)))
            in_maps.append({"x": xp, **shared})
    return in_maps


def assemble_output(results):
    y = np.empty((B, C, N), np.float32)
    for core, res in enumerate(results):
        b, half = core // 2, core % 2
        y[b][:, half * NQ:(half + 1) * NQ] = res["y"]
    return y.reshape(B, C, HW, HW)


def _patch_ldw_opt():
    from concourse import bass_utils
    if getattr(bass_utils, "_ldw_patched", False):
        return
    orig = bass_utils.run_command

    def patched(argv, **kw):
        argv = ["--enable-ldw-opt=true" if a == "--enable-ldw-opt=false" else a
                for a in argv]
        return orig(argv, **kw)

    bass_utils.run_command = patched
    bass_utils._ldw_patched = True


def kernel(**inputs):
    from concourse.bass_utils import run_bass_kernel_spmd

    if LDW_OPT:
        _patch_ldw_opt()
    nc = get_program()
    in_maps = make_in_maps(inputs)
    out = run_bass_kernel_spmd(nc, in_maps, core_ids=list(range(8)))
    return assemble_output(out.results)


if __name__ == "__main__":
    get_program()
    print("built ok")


# revision 10
# speedup vs baseline: 1.0892x; 1.0325x over previous
"""Trainium2 Bass kernel for nn_AttnAware (pixnorm->conv1x1 q/k attention + ResnetBlock).

Sharding: 8 cores = 4 batches x 2 query-halves. Each core receives its batch's
x [256, 4096] with pixel columns rotated so that its 2048 query pixels are the
first 2048 columns (attention is permutation-invariant over keys, and all
other ops are per-pixel). Single SPMD program, no collectives.

Per-core data layout: channels on partitions, pixels on free axis.
All PE matmuls run in fp16 (1 cycle/row streaming AND 1 cycle/row LDWEIGHTS;
conv weights are converted to fp16 host-side). Attention works in the S^T
orientation: S^T[j,i] tiles [128 keys, i-chunk] computed as k_block^T @ q,
exp on ACT (scale fused) writing fp16 P tiles, O^T accumulated as
V^T_block^T @ P^T (V^T pre-transposed once per head on the PE from an fp16
copy of x). The softmax denominator accumulates fp16 P tiles on DVE (2x
rate), folds to a [1,IW] row via a ones-column matmul, takes 1/D with the
fast approximate DVE reciprocal, broadcasts across partitions on the Pool
engine and normalizes O inline per (head, i-pass). Row->tile broadcasts of
the pixnorm scales also go through the Pool engine (partition_broadcast)
instead of PE ones-row matmuls.
"""

import math
from contextlib import ExitStack

import numpy as np

import concourse.bass as bass
import concourse.mybir as mybir
import concourse.tile as tile
from concourse import bacc
from concourse.masks import make_identity

# ---------------- problem constants (hardcoded per contract) ----------------
B = 4
C = 256
HW = 64
N = HW * HW              # 4096 pixels
NQ = N // 2              # 2048 query pixels per core
NH = 2
HD = C // NH             # 128
CT = C // 128            # 2 channel tiles
C2T = 2 * C // 128       # 4 channel tiles for cat
JB = N // 128            # 32 key blocks
ATT_SCALE = HD ** -0.5
RATIO = 1.0 / (1.0 + 1e-8)   # PartialConv mask ratio (== 1.0f in fp32)
EPS = 1e-8
ISQ2 = 1.0 / math.sqrt(2.0)

# ---------------- tuning knobs ----------------
IW = 1024                # i-columns per attention pass (PSUM S tile width)
LDW_OPT = False          # walrus ldw-opt is incompatible with fp16 ldweights

f32 = mybir.dt.float32
f32r = mybir.dt.float32r
f16 = mybir.dt.float16
AF = mybir.ActivationFunctionType
OP = mybir.AluOpType


def r(ap):
    return ap.bitcast(f32r)


def build_program():
    nc = bacc.Bacc("TRN2", target_bir_lowering=False, debug=False)

    # register the pixnorm epsilon as a const AP usable as an ACT bias
    _eps_t = nc.alloc_sbuf_tensor(f"const-float32-{EPS}", [128, 1], f32)
    nc.gpsimd.memset(_eps_t.ap(), EPS)
    nc.const_aps.aps[(f32, EPS)] = _eps_t.ap()
    nc.all_engine_barrier()

    d = {}
    d["x"] = nc.dram_tensor("x", (C, N), f32, kind="ExternalInput").ap()
    for nm, rows in [("wqT", C), ("wkT", C), ("wsT", 2 * C), ("w1T", 2 * C),
                     ("w2T", C)]:
        d[nm] = nc.dram_tensor(nm, (rows, C), f16, kind="ExternalInput").ap()
    for nm, rows in [("bq", C), ("bk", C), ("b1", C), ("bsc", C), ("aq", C),
                     ("ak", C), ("ar1", 2 * C), ("ar2", C)]:
        d[nm] = nc.dram_tensor(nm, (rows, 1), f32, kind="ExternalInput").ap()
    d["y"] = nc.dram_tensor("y", (C, NQ), f32, kind="ExternalOutput").ap()

    with tile.TileContext(nc) as tc:
        _body(tc, nc, d)
    nc.compile()
    return nc


def _body(tc, nc, d):
    x_d, y_d = d["x"], d["y"]

    with ExitStack() as top:
        const = top.enter_context(tc.tile_pool(name="const", bufs=1))
        wts = top.enter_context(tc.tile_pool(name="wts", bufs=1))

        ident16 = const.tile([128, 128], f16, tag="ident16", name="ident16")
        make_identity(nc, ident16[:])
        ones_col16 = const.tile([128, 1], f16, tag="ones_col16", name="ones_col16")
        nc.vector.memset(ones_col16[:], 1.0)

        def load_split(name, n_tiles, width, dt=f32):
            ts = []
            for i in range(n_tiles):
                t = wts.tile([128, width], dt, tag=f"{name}{i}", name=f"{name}{i}")
                nc.sync.dma_start(t[:], d[name][i * 128:(i + 1) * 128, :])
                ts.append(t)
            return ts

        # tensors that live into phase C
        with tc.tile_pool(name="oout", bufs=1) as oout:
            osb = [oout.tile([128, NQ], f16, tag=f"o{h}", name=f"o{h}")
                   for h in range(NH)]
            x16 = [oout.tile([128, N], f16, tag=f"x16_{h}", name=f"x16_{h}")
                   for h in range(NH)]

            # kqv: tensors that live from phase A through attention; closed
            # explicitly before the ResnetBlock pools open to reuse SBUF
            kqv_stack = ExitStack()
            kqv = kqv_stack.enter_context(tc.tile_pool(name="kqv", bufs=1))
            vt = [kqv.tile([128, N], f16, tag=f"vt{h}", name=f"vt{h}") for h in range(NH)]
            kt = [kqv.tile([128, N], f16, tag=f"k{h}", name=f"k{h}") for h in range(NH)]
            qt = [kqv.tile([128, NQ], f16, tag=f"q{h}", name=f"q{h}") for h in range(NH)]

            # x DMA first (critical path), in 1024-col chunks so downstream
            # work starts as soon as the first chunks land; xt (f32) is
            # phase-A-only (front pool closes after the casts/products)
            front_stack = ExitStack()
            front = front_stack.enter_context(tc.tile_pool(name="front", bufs=1))
            xt = [front.tile([128, N], f32, tag=f"x{ct}", name=f"x{ct}")
                  for ct in range(CT)]
            for q in range(N // 1024):
                for ct in range(CT):
                    nc.sync.dma_start(
                        xt[ct][:, q * 1024:(q + 1) * 1024],
                        x_d[ct * 128:(ct + 1) * 128, q * 1024:(q + 1) * 1024])

            wqT = load_split("wqT", CT, C, f16)
            wkT = load_split("wkT", CT, C, f16)
            wsT = load_split("wsT", C2T, C, f16)
            w1T = load_split("w1T", C2T, C, f16)
            w2T = load_split("w2T", CT, C, f16)
            bq = load_split("bq", CT, 1)
            bk = load_split("bk", CT, 1)
            b1 = load_split("b1", CT, 1)
            bsc = load_split("bsc", CT, 1)
            aq = load_split("aq", CT, 1)
            ak = load_split("ak", CT, 1)
            ar1 = load_split("ar1", C2T, 1)
            ar2 = load_split("ar2", CT, 1)

            # =========== Phase A ===========
            with (
                tc.tile_pool(name="gtmp", bufs=6) as gtmp,
                tc.tile_pool(name="frow", bufs=2) as frow,
                tc.tile_pool(name="bcp", bufs=3) as bcp,
                tc.tile_pool(name="psA", bufs=2, space="PSUM") as psA,
                tc.tile_pool(name="psArow", bufs=2, space="PSUM") as psArow,
            ):
                # fp16 copy of x (DVE), per head tile
                for q in range(N // 1024):
                    for h in range(NH):
                        nc.vector.tensor_copy(
                            x16[h][:, q * 1024:(q + 1) * 1024],
                            xt[h][:, q * 1024:(q + 1) * 1024])

                # V^T per head: PE transpose of fp16 x, 4 blocks per PSUM slot
                for h in range(NH):
                    for qb in range(JB // 4):
                        tp = psA.tile([128, 512], f16, tag="tp", name="tp")
                        for rr in range(4):
                            jb = qb * 4 + rr
                            nc.tensor.transpose(
                                tp[:, rr * 128:(rr + 1) * 128],
                                x16[h][:, jb * 128:(jb + 1) * 128], ident16[:])
                        nc.vector.tensor_copy(vt[h][:, qb * 512:(qb + 1) * 512], tp[:])

                # pixelnorm stats: ssum_c x^2 -> inv = exp(-0.5*ln(ssum/C+eps)),
                # per 512-column chunk; inv lands in an fp16 row, then Pool
                # broadcasts it across partitions.
                def inv_chunk(cc):
                    sqc = []
                    for ct in range(CT):
                        t = gtmp.tile([128, 512], f16, tag="g", name="sqch")
                        nc.gpsimd.tensor_tensor(
                            t[:], x16[ct][:, cc * 512:(cc + 1) * 512],
                            x16[ct][:, cc * 512:(cc + 1) * 512], op=OP.mult)
                        sqc.append(t)
                    ss = psArow.tile([1, 512], f32, tag="ssum", name="ssum")
                    for ct in range(CT):
                        nc.tensor.matmul(ss[:], ones_col16[:], sqc[ct][:],
                                         start=(ct == 0), stop=(ct == CT - 1))
                    lt = frow.tile([1, 512], f32, tag="lnt", name="lnt")
                    nc.scalar.activation(lt[:], ss[:], AF.Ln, bias=EPS, scale=1.0 / C)
                    iv = frow.tile([1, 512], f16, tag="inv", name="inv", bufs=8)
                    nc.scalar.activation(iv[:], lt[:], AF.Exp, scale=-0.5)
                    return iv

                # batch all pixelnorm stats first (single lnexp table residency)
                all_inv = [inv_chunk(cc) for cc in range(N // 512)]

                # per-chunk: bcast inv (Pool), xb = x*inv, gelu per conv, matmuls
                def conv_block(cc, convs):
                    bc = bcp.tile([128, 512], f16, tag="bc", name="bc")
                    nc.gpsimd.partition_broadcast(bc[:], all_inv[cc][:])
                    xb = []
                    for ct in range(CT):
                        t = gtmp.tile([128, 512], f16, tag="g", name="xb")
                        nc.vector.tensor_tensor(
                            t[:], x16[ct][:, cc * 512:(cc + 1) * 512], bc[:],
                            op=OP.mult)
                        xb.append(t)
                    for (wT, alpha, bias, out_tiles, oc0) in convs:
                        gchunks = []
                        for ct in range(CT):
                            g = gtmp.tile([128, 512], f16, tag="g", name="g")
                            nc.scalar.activation(g[:], xb[ct][:],
                                                 AF.Gelu, scale=alpha[ct][:])
                            gchunks.append(g)
                        osl = slice(oc0 + (cc % 4) * 512, oc0 + (cc % 4 + 1) * 512)
                        for mo in range(CT):
                            ps = psA.tile([128, 512], f32, tag="conv", name="conv")
                            for kc in range(CT):
                                nc.tensor.matmul(ps[:],
                                                 wT[kc][:, mo * 128:(mo + 1) * 128],
                                                 gchunks[kc][:],
                                                 start=(kc == 0), stop=(kc == CT - 1))
                            nc.vector.tensor_scalar(out_tiles[mo][:, osl],
                                                    ps[:], bias[mo][:], None, op0=OP.add)

                for cc in range(NQ // 512):
                    conv_block(cc, [(wqT, aq, bq, qt, 0), (wkT, ak, bk, kt, 0)])
                for cc in range(NQ // 512, N // 512):
                    conv_block(cc, [(wkT, ak, bk, kt, NQ)])

            front_stack.close()  # xt no longer needed (phase C uses x16)

            # =========== Phase B: attention ===========
            with (
                tc.tile_pool(name="psS", bufs=3, space="PSUM") as psS,
                tc.tile_pool(name="psO", bufs=1, space="PSUM") as psO,
                tc.tile_pool(name="pexp", bufs=3) as pexp,
                tc.tile_pool(name="dacc", bufs=2) as dacc_pool,
                tc.tile_pool(name="drow", bufs=2) as drow_pool,
            ):
                NR = IW // 512
                for h in range(NH):
                    for ip in range(NQ // IW):
                        i0 = ip * IW
                        o_ps = psO.tile([128, IW], f32, tag="o", name="o")
                        dac = dacc_pool.tile([128, IW], f16, tag="dacc", name="dacc")
                        for jb in range(JB):
                            s_ps = psS.tile([128, IW], f32, tag="s", name="s")
                            for rr in range(NR):
                                nc.tensor.matmul(
                                    s_ps[:, rr * 512:(rr + 1) * 512],
                                    kt[h][:, jb * 128:(jb + 1) * 128],
                                    qt[h][:, i0 + rr * 512:i0 + (rr + 1) * 512],
                                    start=True, stop=True)
                            p_sb = pexp.tile([128, IW], f16, tag="p", name="p")
                            nc.scalar.activation(p_sb[:], s_ps[:],
                                                 AF.Exp, scale=ATT_SCALE)
                            for rr in range(NR):
                                nc.tensor.matmul(
                                    o_ps[:, rr * 512:(rr + 1) * 512],
                                    vt[h][:, jb * 128:(jb + 1) * 128],
                                    p_sb[:, rr * 512:(rr + 1) * 512],
                                    start=(jb == 0), stop=(jb == JB - 1))
                            if jb == 0:
                                nc.vector.tensor_copy(dac[:], p_sb[:])
                            else:
                                nc.vector.tensor_tensor(dac[:], dac[:],
                                                        p_sb[:], op=OP.add)
                        # fold dacc over partitions -> D row, then 1/D on DVE
                        d_ps = psS.tile([1, IW], f32, tag="s", name="d")
                        for rr in range(NR):
                            nc.tensor.matmul(
                                d_ps[:, rr * 512:(rr + 1) * 512],
                                ones_col16[:],
                                dac[:, rr * 512:(rr + 1) * 512],
                                start=True, stop=True)
                        dinv = drow_pool.tile([1, IW], f32, tag="dinv", name="dinv")
                        nc.vector.reciprocal_approx_fast(out=dinv[:], in_=d_ps[:])
                        # bcast 1/D across partitions (Pool) and normalize O
                        bc = drow_pool.tile([128, IW], f32, tag="bcD", name="bcD")
                        nc.gpsimd.partition_broadcast(bc[:], dinv[:])
                        nc.vector.tensor_tensor(
                            osb[h][:, i0:i0 + IW], o_ps[:], bc[:], op=OP.mult)

            # kqv pool (k/q/vt) closes here; back pool reuses its space
            kqv_stack.close()

            # ======= Phase C: ResnetBlock =======
            with (
                tc.tile_pool(name="back", bufs=1) as back,
                tc.tile_pool(name="brow", bufs=2) as brow,
                tc.tile_pool(name="tmp", bufs=4) as tmp,
                tc.tile_pool(name="psB2", bufs=3, space="PSUM") as psB2,
                tc.tile_pool(name="psBrow2", bufs=2, space="PSUM") as psBrow2,
            ):
                cat = [osb[0], osb[1], x16[0], x16[1]]  # fp16, sliced to [:, :NQ]

                def stats(tiles, nch, tag):
                    # inv row [1, NQ] fp16 + Pool broadcast to [128, NQ] fp16
                    ivrow = brow.tile([1, NQ], f16, tag=f"ivrow{tag}",
                                      name=f"ivrow{tag}")
                    for cc in range(NQ // 512):
                        ss = psBrow2.tile([1, 512], f32, tag="ssum", name="ssum")
                        for i, t in enumerate(tiles):
                            nc.tensor.matmul(ss[:], ones_col16[:],
                                             t[:, cc * 512:(cc + 1) * 512],
                                             start=(i == 0),
                                             stop=(i == len(tiles) - 1))
                        lt = brow.tile([1, 512], f32, tag="lnt", name="lnt", bufs=4)
                        nc.scalar.activation(lt[:], ss[:], AF.Ln, bias=EPS,
                                             scale=1.0 / nch)
                        nc.scalar.activation(ivrow[:, cc * 512:(cc + 1) * 512],
                                             lt[:], AF.Exp, scale=-0.5)
                    bc = back.tile([128, NQ], f16, tag=f"bc{tag}", name=f"bc{tag}")
                    nc.gpsimd.partition_broadcast(bc[:], ivrow[:])
                    return bc

                # r1 stats over 512 channels of cat
                sqc = []
                for ct in range(C2T):
                    t = tmp.tile([128, NQ], f16, tag="sqc", name="sqc")
                    nc.gpsimd.tensor_tensor(t[:], cat[ct][:, :NQ],
                                            cat[ct][:, :NQ], op=OP.mult)
                    sqc.append(t)
                bc1 = stats(sqc, 2 * C, "r1")

                # x_short (scaled by 1/sqrt2; bias (bs+b2)/sqrt2)
                xs = [back.tile([128, NQ], f32, tag=f"xs{mo}", name=f"xs{mo}") for mo in range(CT)]
                for mo in range(CT):
                    for cc in range(NQ // 512):
                        ps = psB2.tile([128, 512], f32, tag="conv", name="conv")
                        for kc in range(C2T):
                            nc.tensor.matmul(
                                ps[:], wsT[kc][:, mo * 128:(mo + 1) * 128],
                                cat[kc][:, cc * 512:(cc + 1) * 512],
                                start=(kc == 0), stop=(kc == C2T - 1))
                        nc.vector.tensor_scalar(
                            xs[mo][:, cc * 512:(cc + 1) * 512], ps[:],
                            RATIO * ISQ2, bsc[mo][:], op0=OP.mult, op1=OP.add)

                # gr1 = gelu(alpha_r1 * cat * invr1)
                gr1 = []
                for ct in range(C2T):
                    cn = tmp.tile([128, NQ], f16, tag="sqc", name="sqc")
                    nc.vector.tensor_tensor(cn[:], cat[ct][:, :NQ], bc1[:], op=OP.mult)
                    t = back.tile([128, NQ], f16, tag=f"gr1{ct}", name=f"gr1{ct}")
                    nc.scalar.activation(t[:], cn[:], AF.Gelu, scale=ar1[ct][:])
                    gr1.append(t)

                # h1 = W1 @ gr1 * ratio + b1
                h1 = [back.tile([128, NQ], f16, tag=f"h1{mo}", name=f"h1{mo}") for mo in range(CT)]
                for mo in range(CT):
                    for cc in range(NQ // 512):
                        ps = psB2.tile([128, 512], f32, tag="conv", name="conv")
                        for kc in range(C2T):
                            nc.tensor.matmul(
                                ps[:], w1T[kc][:, mo * 128:(mo + 1) * 128],
                                gr1[kc][:, cc * 512:(cc + 1) * 512],
                                start=(kc == 0), stop=(kc == C2T - 1))
                        nc.vector.tensor_scalar(
                            h1[mo][:, cc * 512:(cc + 1) * 512], ps[:],
                            RATIO, b1[mo][:], op0=OP.mult, op1=OP.add)

                # r2 stats over h1
                sqh = []
                for ct in range(CT):
                    t = tmp.tile([128, NQ], f16, tag="sqc", name="sqc")
                    nc.gpsimd.tensor_tensor(t[:], h1[ct][:], h1[ct][:], op=OP.mult)
                    sqh.append(t)
                bc2 = stats(sqh, C, "r2")

                # gr2 = gelu(alpha_r2 * h1 * invr2)
                gr2 = []
                for ct in range(CT):
                    hn = tmp.tile([128, NQ], f16, tag="sqc", name="sqc")
                    nc.vector.tensor_tensor(hn[:], h1[ct][:], bc2[:], op=OP.mult)
                    t = back.tile([128, NQ], f16, tag=f"gr2{ct}", name=f"gr2{ct}")
                    nc.scalar.activation(t[:], hn[:], AF.Gelu, scale=ar2[ct][:])
                    gr2.append(t)

                # y = W2 @ gr2 * ratio/sqrt2 + xs
                for mo in range(CT):
                    yt = back.tile([128, NQ], f32, tag=f"yt{mo}", name=f"yt{mo}")
                    for cc in range(NQ // 512):
                        ps = psB2.tile([128, 512], f32, tag="conv", name="conv")
                        for kc in range(CT):
                            nc.tensor.matmul(
                                ps[:], w2T[kc][:, mo * 128:(mo + 1) * 128],
                                gr2[kc][:, cc * 512:(cc + 1) * 512],
                                start=(kc == 0), stop=(kc == CT - 1))
                        nc.vector.scalar_tensor_tensor(
                            yt[:, cc * 512:(cc + 1) * 512], ps[:], RATIO * ISQ2,
                            xs[mo][:, cc * 512:(cc + 1) * 512],
                            op0=OP.mult, op1=OP.add)
                    nc.sync.dma_start(y_d[mo * 128:(mo + 1) * 128, :], yt[:])


_PROGRAM = None


def get_program():
    global _PROGRAM
    if _PROGRAM is None:
        _PROGRAM = build_program()
    return _PROGRAM


def make_in_maps(inputs):
    x = np.asarray(inputs["x"], np.float32).reshape(B, C, N)
    col = lambda v, n: np.ascontiguousarray(np.asarray(v, np.float32).reshape(n, 1))
    tr16 = lambda w: np.ascontiguousarray(np.asarray(w, np.float32).T).astype(np.float16)
    shared = {
        "wqT": tr16(inputs["Wq"]), "wkT": tr16(inputs["Wk"]),
        "wsT": tr16(inputs["Ws"]), "w1T": tr16(inputs["W1"]),
        "w2T": tr16(inputs["W2"]),
        "bq": col(inputs["bq"], C), "bk": col(inputs["bk"], C),
        "b1": col(inputs["b1"], C),
        "bsc": ((col(inputs["bs"], C).astype(np.float64) +
                 col(inputs["b2"], C).astype(np.float64)) * ISQ2).astype(np.float32),
        "aq": col(inputs["alpha_q"], C), "ak": col(inputs["alpha_k"], C),
        "ar1": col(inputs["alpha_r1"], 2 * C), "ar2": col(inputs["alpha_r2"], C),
    }
    in_maps = []
    for b in range(B):
        for half in range(2):
            xp = (np.ascontiguousarray(x[b]) if half == 0
                  else np.ascontiguousarray(np.roll(x[b], -NQ, axis=1)))
            in_maps.append({"x": xp, **shared})
    return in_maps


def assemble_output(results):
    y = np.empty((B, C, N), np.float32)
    for core, res in enumerate(results):
        b, half = core // 2, core % 2
        y[b][:, half * NQ:(half + 1) * NQ] = res["y"]
    return y.reshape(B, C, HW, HW)


def _patch_ldw_opt():
    from concourse import bass_utils
    if getattr(bass_utils, "_ldw_patched", False):
        return
    orig = bass_utils.run_command

    def patched(argv, **kw):
        argv = ["--enable-ldw-opt=true" if a == "--enable-ldw-opt=false" else a
                for a in argv]
        return orig(argv, **kw)

    bass_utils.run_command = patched
    bass_utils._ldw_patched = True


def kernel(**inputs):
    from concourse.bass_utils import run_bass_kernel_spmd

    if LDW_OPT:
        _patch_ldw_opt()
    nc = get_program()
    in_maps = make_in_maps(inputs)
    out = run_bass_kernel_spmd(nc, in_maps, core_ids=list(range(8)))
    return assemble_output(out.results)


if __name__ == "__main__":
    get_program()
    print("built ok")


# revision 14
# speedup vs baseline: 1.1157x; 1.0243x over previous
"""Trainium2 Bass kernel for nn_AttnAware (pixnorm->conv1x1 q/k attention + ResnetBlock).

Sharding: 8 cores = 4 batches x 2 query-halves. Each core receives its batch's
x [256, 4096] with pixel columns rotated so that its 2048 query pixels are the
first 2048 columns (attention is permutation-invariant over keys, and all
other ops are per-pixel). Single SPMD program, no collectives.

Per-core data layout: channels on partitions, pixels on free axis.
All PE matmuls run in 16/8-bit (1 cycle/row streaming and cheap LDWEIGHTS;
conv weights are converted to fp16 host-side and shipped in one packed DMA).
Attention works in the S^T orientation: S^T[j,i] tiles [128 keys, i-chunk]
computed as k_block^T @ q in fp16, exp on ACT (scale fused) writing fp8e4 P
pair-tiles, O^T accumulated with fp8 DoubleRow matmuls (two key blocks per
instruction, 0.5 cycles/row) against V^T pre-transposed per head on the PE.
The softmax denominator is accumulated on the PE as fp8 DoubleRow
ones-matmuls over the same P tiles, 1/D comes from the fast approximate DVE
reciprocal, is broadcast across partitions on the Pool engine, and O is
normalized inline per (head, i-pass). Pixelnorm row->tile broadcasts also go
through the Pool engine (partition_broadcast) instead of PE matmuls.
"""

import math
from contextlib import ExitStack

import numpy as np

import concourse.bass as bass
import concourse.mybir as mybir
import concourse.tile as tile
from concourse import bacc
from concourse.masks import make_identity

# ---------------- problem constants (hardcoded per contract) ----------------
B = 4
C = 256
HW = 64
N = HW * HW              # 4096 pixels
NQ = N // 2              # 2048 query pixels per core
NH = 2
HD = C // NH             # 128
CT = C // 128            # 2 channel tiles
C2T = 2 * C // 128       # 4 channel tiles for cat
JB = N // 128            # 32 key blocks
ATT_SCALE = HD ** -0.5
RATIO = 1.0 / (1.0 + 1e-8)   # PartialConv mask ratio (== 1.0f in fp32)
EPS = 1e-8
ISQ2 = 1.0 / math.sqrt(2.0)

# packed weight tile order (14 x [128, 256] fp16 column-tiles)
W_ORDER = [("wqT", CT), ("wkT", CT), ("wsT", C2T), ("w1T", C2T), ("w2T", CT)]
NWT = sum(n for _, n in W_ORDER)
# packed bias/alpha column order (18 x [128, 1] f32)
B_ORDER = [("bq", CT), ("bk", CT), ("b1", CT), ("bsc", CT), ("aq", CT),
           ("ak", CT), ("ar1", C2T), ("ar2", CT)]
NBC = sum(n for _, n in B_ORDER)

# ---------------- tuning knobs ----------------
IW = 1024                # i-columns per attention pass (PSUM S tile width)
LDW_OPT = False          # walrus ldw-opt is incompatible with 16/8-bit ldweights

f32 = mybir.dt.float32
f32r = mybir.dt.float32r
f16 = mybir.dt.float16
f8 = mybir.dt.float8e4
AF = mybir.ActivationFunctionType
OP = mybir.AluOpType
DR = mybir.MatmulPerfMode.DoubleRow


def build_program():
    nc = bacc.Bacc("TRN2", target_bir_lowering=False, debug=False)

    # register the pixnorm epsilon as a const AP usable as an ACT bias
    _eps_t = nc.alloc_sbuf_tensor(f"const-float32-{EPS}", [128, 1], f32)
    nc.gpsimd.memset(_eps_t.ap(), EPS)
    nc.const_aps.aps[(f32, EPS)] = _eps_t.ap()
    nc.all_engine_barrier()

    d = {}
    d["x"] = nc.dram_tensor("x", (C, N), f32, kind="ExternalInput").ap()
    d["wpack"] = nc.dram_tensor("wpack", (128, NWT * 256), f16,
                                kind="ExternalInput").ap()
    d["bpack"] = nc.dram_tensor("bpack", (128, NBC), f32,
                                kind="ExternalInput").ap()
    d["y"] = nc.dram_tensor("y", (C, NQ), f32, kind="ExternalOutput").ap()

    with tile.TileContext(nc) as tc:
        _body(tc, nc, d)
    nc.compile()
    return nc


def _body(tc, nc, d):
    x_d, y_d = d["x"], d["y"]

    with ExitStack() as top:
        const = top.enter_context(tc.tile_pool(name="const", bufs=1))
        wts = top.enter_context(tc.tile_pool(name="wts", bufs=1))

        ident16 = const.tile([128, 128], f16, tag="ident16", name="ident16")
        make_identity(nc, ident16[:])
        ones_col16 = const.tile([128, 1], f16, tag="ones_col16", name="ones_col16")
        nc.vector.memset(ones_col16[:], 1.0)
        ones_dr8 = const.tile([128, 2, 32], f8, tag="ones_dr8", name="ones_dr8")
        nc.vector.memset(ones_dr8[:], 1.0)

        # tensors that live into phase C
        with tc.tile_pool(name="oout", bufs=1) as oout:
            osb = [oout.tile([128, NQ], f16, tag=f"o{h}", name=f"o{h}")
                   for h in range(NH)]
            x16 = [oout.tile([128, N], f16, tag=f"x16_{h}", name=f"x16_{h}")
                   for h in range(NH)]

            # kqv: tensors that live from phase A through attention; closed
            # explicitly before the ResnetBlock pools open to reuse SBUF
            kqv_stack = ExitStack()
            kqv = kqv_stack.enter_context(tc.tile_pool(name="kqv", bufs=1))
            vt = [kqv.tile([128, N], f8, tag=f"vt{h}", name=f"vt{h}") for h in range(NH)]
            kt = [kqv.tile([128, N], f16, tag=f"k{h}", name=f"k{h}") for h in range(NH)]
            qt = [kqv.tile([128, NQ], f16, tag=f"q{h}", name=f"q{h}") for h in range(NH)]

            # x DMA first (critical path): 4 transfers, first pixel-half first
            front_stack = ExitStack()
            front = front_stack.enter_context(tc.tile_pool(name="front", bufs=1))
            xt = [front.tile([128, N], f32, tag=f"x{ct}", name=f"x{ct}")
                  for ct in range(CT)]
            for half in range(2):
                for ct in range(CT):
                    nc.sync.dma_start(
                        xt[ct][:, half * NQ:(half + 1) * NQ],
                        x_d[ct * 128:(ct + 1) * 128, half * NQ:(half + 1) * NQ])

            # packed weights (one DMA) + packed biases/alphas (one DMA)
            wtile = wts.tile([128, NWT * 256], f16, tag="wpack", name="wpack")
            nc.sync.dma_start(wtile[:], d["wpack"])
            btile = wts.tile([128, NBC], f32, tag="bpack", name="bpack")
            nc.sync.dma_start(btile[:], d["bpack"])

            wv = {}
            off = 0
            for nm, n in W_ORDER:
                wv[nm] = [wtile[:, (off + i) * 256:(off + i + 1) * 256]
                          for i in range(n)]
                off += n
            bv = {}
            off = 0
            for nm, n in B_ORDER:
                bv[nm] = [btile[:, off + i:off + i + 1] for i in range(n)]
                off += n
            wqT, wkT, wsT, w1T, w2T = (wv[nm] for nm, _ in W_ORDER)
            bq, bk, b1, bsc, aq, ak, ar1, ar2 = (bv[nm] for nm, _ in B_ORDER)

            # =========== Phase A ===========
            with (
                tc.tile_pool(name="gtmp", bufs=6) as gtmp,
                tc.tile_pool(name="frow", bufs=2) as frow,
                tc.tile_pool(name="bcp", bufs=3) as bcp,
                tc.tile_pool(name="psA", bufs=2, space="PSUM") as psA,
                tc.tile_pool(name="psArow", bufs=2, space="PSUM") as psArow,
            ):
                # fp16 copy of x (DVE), per head tile
                for q in range(N // 1024):
                    for h in range(NH):
                        nc.vector.tensor_copy(
                            x16[h][:, q * 1024:(q + 1) * 1024],
                            xt[h][:, q * 1024:(q + 1) * 1024])

                # V^T per head: PE transpose of fp16 x, 4 blocks per PSUM slot,
                # copied out with a cast to fp8 (layout [(jb) d] flat == the
                # [jb/2, 2, d] DoubleRow view)
                for h in range(NH):
                    for qb in range(JB // 4):
                        tp = psA.tile([128, 512], f16, tag="tp", name="tp")
                        for rr in range(4):
                            jb = qb * 4 + rr
                            nc.tensor.transpose(
                                tp[:, rr * 128:(rr + 1) * 128],
                                x16[h][:, jb * 128:(jb + 1) * 128], ident16[:])
                        nc.vector.tensor_copy(vt[h][:, qb * 512:(qb + 1) * 512], tp[:])

                # pixelnorm stats: ssum_c x^2 -> inv = exp(-0.5*ln(ssum/C+eps)),
                # per 512-column chunk; inv lands in an fp16 row, then Pool
                # broadcasts it across partitions.
                def inv_chunk(cc):
                    sqc = []
                    for ct in range(CT):
                        t = gtmp.tile([128, 512], f16, tag="g", name="sqch")
                        nc.gpsimd.tensor_tensor(
                            t[:], x16[ct][:, cc * 512:(cc + 1) * 512],
                            x16[ct][:, cc * 512:(cc + 1) * 512], op=OP.mult)
                        sqc.append(t)
                    ss = psArow.tile([1, 512], f32, tag="ssum", name="ssum")
                    for ct in range(CT):
                        nc.tensor.matmul(ss[:], ones_col16[:], sqc[ct][:],
                                         start=(ct == 0), stop=(ct == CT - 1))
                    lt = frow.tile([1, 512], f32, tag="lnt", name="lnt")
                    nc.scalar.activation(lt[:], ss[:], AF.Ln, bias=EPS, scale=1.0 / C)
                    iv = frow.tile([1, 512], f16, tag="inv", name="inv", bufs=8)
                    nc.scalar.activation(iv[:], lt[:], AF.Exp, scale=-0.5)
                    return iv

                # batch all pixelnorm stats first (single lnexp table residency)
                all_inv = [inv_chunk(cc) for cc in range(N // 512)]

                # per-chunk: bcast inv (Pool), xb = x*inv, gelu per conv, matmuls
                def conv_block(cc, convs):
                    bc = bcp.tile([128, 512], f16, tag="bc", name="bc")
                    nc.gpsimd.partition_broadcast(bc[:], all_inv[cc][:])
                    xb = []
                    for ct in range(CT):
                        t = gtmp.tile([128, 512], f16, tag="g", name="xb")
                        nc.vector.tensor_tensor(
                            t[:], x16[ct][:, cc * 512:(cc + 1) * 512], bc[:],
                            op=OP.mult)
                        xb.append(t)
                    for (wT, alpha, bias, out_tiles, oc0) in convs:
                        gchunks = []
                        for ct in range(CT):
                            g = gtmp.tile([128, 512], f16, tag="g", name="g")
                            nc.scalar.activation(g[:], xb[ct][:],
                                                 AF.Gelu, scale=alpha[ct])
                            gchunks.append(g)
                        osl = slice(oc0 + (cc % 4) * 512, oc0 + (cc % 4 + 1) * 512)
                        for mo in range(CT):
                            ps = psA.tile([128, 512], f32, tag="conv", name="conv")
                            for kc in range(CT):
                                nc.tensor.matmul(ps[:],
                                                 wT[kc][:, mo * 128:(mo + 1) * 128],
                                                 gchunks[kc][:],
                                                 start=(kc == 0), stop=(kc == CT - 1))
                            nc.vector.tensor_scalar(out_tiles[mo][:, osl],
                                                    ps[:], bias[mo], None, op0=OP.add)

                for cc in range(NQ // 512):
                    conv_block(cc, [(wqT, aq, bq, qt, 0), (wkT, ak, bk, kt, 0)])
                for cc in range(NQ // 512, N // 512):
                    conv_block(cc, [(wkT, ak, bk, kt, NQ)])

            front_stack.close()  # xt no longer needed (phase C uses x16)

            # =========== Phase B: attention (fp8 DoubleRow O and D) ==========
            with (
                tc.tile_pool(name="psS", bufs=2, space="PSUM") as psS,
                tc.tile_pool(name="psO", bufs=1, space="PSUM") as psO,
                tc.tile_pool(name="psD", bufs=1, space="PSUM") as psD,
                tc.tile_pool(name="pexp", bufs=3) as pexp,
                tc.tile_pool(name="drow", bufs=2) as drow_pool,
            ):
                NR = IW // 512
                NJP = JB // 2
                for h in range(NH):
                    for ip in range(NQ // IW):
                        i0 = ip * IW
                        o_ps = psO.tile([128, IW], f32, tag="o", name="o")
                        d_ps = psD.tile([32, IW], f32, tag="d", name="d")
                        for jp in range(NJP):
                            p2 = pexp.tile([128, 2, IW], f8, tag="p", name="p")
                            for jbi in range(2):
                                jb = jp * 2 + jbi
                                s_ps = psS.tile([128, IW], f32, tag="s", name="s")
                                for rr in range(NR):
                                    nc.tensor.matmul(
                                        s_ps[:, rr * 512:(rr + 1) * 512],
                                        kt[h][:, jb * 128:(jb + 1) * 128],
                                        qt[h][:, i0 + rr * 512:i0 + (rr + 1) * 512],
                                        start=True, stop=True)
                                nc.scalar.activation(p2[:, jbi, :], s_ps[:],
                                                     AF.Exp, scale=ATT_SCALE)
                            vt_dr = vt[h][:, jp * 256:(jp + 1) * 256].rearrange(
                                "p (j d) -> p j d", j=2)
                            for rr in range(NR):
                                psl = p2[:, :, rr * 512:(rr + 1) * 512]
                                nc.tensor.matmul(
                                    o_ps[:, rr * 512:(rr + 1) * 512],
                                    vt_dr, psl,
                                    start=(jp == 0), stop=(jp == NJP - 1),
                                    perf_mode=DR)
                                nc.tensor.matmul(
                                    d_ps[:, rr * 512:(rr + 1) * 512],
                                    ones_dr8[:], psl,
                                    start=(jp == 0), stop=(jp == NJP - 1),
                                    perf_mode=DR)
                        dinv = drow_pool.tile([1, IW], f32, tag="dinv", name="dinv")
                        nc.vector.reciprocal_approx_fast(out=dinv[:], in_=d_ps[0:1, :])
                        # bcast 1/D across partitions (Pool) and normalize O
                        bc = drow_pool.tile([128, IW], f32, tag="bcD", name="bcD")
                        nc.gpsimd.partition_broadcast(bc[:], dinv[:])
                        nc.vector.tensor_tensor(
                            osb[h][:, i0:i0 + IW], o_ps[:], bc[:], op=OP.mult)

            # kqv pool (k/q/vt) closes here; back pool reuses its space
            kqv_stack.close()

            # ======= Phase C: ResnetBlock (per-512-column pipeline) =======
            with (
                tc.tile_pool(name="back", bufs=1) as back,
                tc.tile_pool(name="brow", bufs=4) as brow,
                tc.tile_pool(name="tmp", bufs=8) as tmp,
                tc.tile_pool(name="psB2", bufs=3, space="PSUM") as psB2,
                tc.tile_pool(name="psBrow2", bufs=2, space="PSUM") as psBrow2,
            ):
                NCC = NQ // 512
                cat = [osb[0], osb[1], x16[0], x16[1]]  # fp16, use [:, :NQ]

                def sl(t, cc):
                    return t[:, cc * 512:(cc + 1) * 512]

                def stats_chunk(tiles, nch, tag, cc):
                    # returns bcast [128,512] fp16 pixnorm scale for chunk cc
                    sq = []
                    for i, t in enumerate(tiles):
                        s = tmp.tile([128, 512], f16, tag="sq", name="sq", bufs=8)
                        eng = nc.gpsimd if i % 2 == 0 else nc.vector
                        eng.tensor_tensor(s[:], sl(t, cc), sl(t, cc), op=OP.mult)
                        sq.append(s)
                    ss = psBrow2.tile([1, 512], f32, tag="ssum", name="ssum")
                    for i, s in enumerate(sq):
                        nc.tensor.matmul(ss[:], ones_col16[:], s[:],
                                         start=(i == 0), stop=(i == len(sq) - 1))
                    lt = brow.tile([1, 512], f32, tag="lnt", name="lnt")
                    nc.scalar.activation(lt[:], ss[:], AF.Ln, bias=EPS,
                                         scale=1.0 / nch)
                    iv = brow.tile([1, 512], f16, tag="iv", name="iv")
                    nc.scalar.activation(iv[:], lt[:], AF.Exp, scale=-0.5)
                    bc = tmp.tile([128, 512], f16, tag=f"bc{tag}",
                                  name=f"bc{tag}", bufs=4)
                    nc.gpsimd.partition_broadcast(bc[:], iv[:])
                    return bc

                # x_short convs first (independent of stats; keeps PE busy)
                xs = [back.tile([128, NQ], f32, tag=f"xs{mo}", name=f"xs{mo}")
                      for mo in range(CT)]
                for cc in range(NCC):
                    for mo in range(CT):
                        ps = psB2.tile([128, 512], f32, tag="conv", name="conv")
                        for kc in range(C2T):
                            nc.tensor.matmul(
                                ps[:], wsT[kc][:, mo * 128:(mo + 1) * 128],
                                sl(cat[kc], cc),
                                start=(kc == 0), stop=(kc == C2T - 1))
                        nc.vector.tensor_scalar(
                            sl(xs[mo], cc), ps[:],
                            RATIO * ISQ2, bsc[mo], op0=OP.mult, op1=OP.add)

                # r1 stats per chunk
                bc1 = [stats_chunk(cat, 2 * C, "r1", cc) for cc in range(NCC)]

                # gr1 = gelu(alpha_r1 * cat * invr1); h1 conv per chunk
                gr1 = [back.tile([128, NQ], f16, tag=f"gr1{ct}", name=f"gr1{ct}")
                       for ct in range(C2T)]
                h1 = [back.tile([128, NQ], f16, tag=f"h1{mo}", name=f"h1{mo}")
                      for mo in range(CT)]
                for cc in range(NCC):
                    for ct in range(C2T):
                        cn = tmp.tile([128, 512], f16, tag="cn", name="cn", bufs=6)
                        nc.vector.tensor_tensor(cn[:], sl(cat[ct], cc), bc1[cc][:],
                                                op=OP.mult)
                        nc.scalar.activation(sl(gr1[ct], cc), cn[:], AF.Gelu,
                                             scale=ar1[ct])
                    for mo in range(CT):
                        ps = psB2.tile([128, 512], f32, tag="conv", name="conv")
                        for kc in range(C2T):
                            nc.tensor.matmul(
                                ps[:], w1T[kc][:, mo * 128:(mo + 1) * 128],
                                sl(gr1[kc], cc),
                                start=(kc == 0), stop=(kc == C2T - 1))
                        nc.vector.tensor_scalar(
                            sl(h1[mo], cc), ps[:],
                            RATIO, b1[mo], op0=OP.mult, op1=OP.add)

                # r2 stats + gr2 + y per chunk
                bc2 = [stats_chunk(h1, C, "r2", cc) for cc in range(NCC)]
                gr2 = [back.tile([128, NQ], f16, tag=f"gr2{ct}", name=f"gr2{ct}")
                       for ct in range(CT)]
                yt = [back.tile([128, NQ], f32, tag=f"yt{mo}", name=f"yt{mo}")
                      for mo in range(CT)]
                for cc in range(NCC):
                    for ct in range(CT):
                        hn = tmp.tile([128, 512], f16, tag="cn", name="hn", bufs=6)
                        nc.vector.tensor_tensor(hn[:], sl(h1[ct], cc), bc2[cc][:],
                                                op=OP.mult)
                        nc.scalar.activation(sl(gr2[ct], cc), hn[:], AF.Gelu,
                                             scale=ar2[ct])
                    for mo in range(CT):
                        ps = psB2.tile([128, 512], f32, tag="conv", name="conv")
                        for kc in range(CT):
                            nc.tensor.matmul(
                                ps[:], w2T[kc][:, mo * 128:(mo + 1) * 128],
                                sl(gr2[kc], cc),
                                start=(kc == 0), stop=(kc == CT - 1))
                        nc.vector.scalar_tensor_tensor(
                            sl(yt[mo], cc), ps[:], RATIO * ISQ2,
                            sl(xs[mo], cc), op0=OP.mult, op1=OP.add)
                    if cc % 2 == 1:
                        for mo in range(CT):
                            nc.sync.dma_start(
                                y_d[mo * 128:(mo + 1) * 128,
                                    (cc - 1) * 512:(cc + 1) * 512],
                                yt[mo][:, (cc - 1) * 512:(cc + 1) * 512])


_PROGRAM = None


def get_program():
    global _PROGRAM
    if _PROGRAM is None:
        _PROGRAM = build_program()
    return _PROGRAM


def make_in_maps(inputs):
    x = np.asarray(inputs["x"], np.float32).reshape(B, C, N)
    col = lambda v, n: np.asarray(v, np.float32).reshape(n, 1)
    tr16 = lambda w: np.ascontiguousarray(
        np.asarray(w, np.float32).T).astype(np.float16)
    wmats = {"wqT": tr16(inputs["Wq"]), "wkT": tr16(inputs["Wk"]),
             "wsT": tr16(inputs["Ws"]), "w1T": tr16(inputs["W1"]),
             "w2T": tr16(inputs["W2"])}
    wpack = np.concatenate(
        [wmats[nm][i * 128:(i + 1) * 128, :]
         for nm, n in W_ORDER for i in range(n)], axis=1)
    bcols = {"bq": col(inputs["bq"], C), "bk": col(inputs["bk"], C),
             "b1": col(inputs["b1"], C),
             "bsc": ((col(inputs["bs"], C).astype(np.float64) +
                      col(inputs["b2"], C).astype(np.float64)) * ISQ2
                     ).astype(np.float32),
             "aq": col(inputs["alpha_q"], C), "ak": col(inputs["alpha_k"], C),
             "ar1": col(inputs["alpha_r1"], 2 * C),
             "ar2": col(inputs["alpha_r2"], C)}
    bpack = np.concatenate(
        [bcols[nm][i * 128:(i + 1) * 128, :]
         for nm, n in B_ORDER for i in range(n)], axis=1)
    shared = {"wpack": np.ascontiguousarray(wpack),
              "bpack": np.ascontiguousarray(bpack.astype(np.float32))}
    in_maps = []
    for b in range(B):
        for half in range(2):
            xp = (np.ascontiguousarray(x[b]) if half == 0
                  else np.ascontiguousarray(np.roll(x[b], -NQ, axis=1)))
            in_maps.append({"x": xp, **shared})
    return in_maps


def assemble_output(results):
    y = np.empty((B, C, N), np.float32)
    for core, res in enumerate(results):
        b, half = core // 2, core % 2
        y[b][:, half * NQ:(half + 1) * NQ] = res["y"]
    return y.reshape(B, C, HW, HW)


def _patch_ldw_opt():
    from concourse import bass_utils
    if getattr(bass_utils, "_ldw_patched", False):
        return
    orig = bass_utils.run_command

    def patched(argv, **kw):
        argv = ["--enable-ldw-opt=true" if a == "--enable-ldw-opt=false" else a
                for a in argv]
        return orig(argv, **kw)

    bass_utils.run_command = patched
    bass_utils._ldw_patched = True


def kernel(**inputs):
    from concourse.bass_utils import run_bass_kernel_spmd

    if LDW_OPT:
        _patch_ldw_opt()
    nc = get_program()
    in_maps = make_in_maps(inputs)
    out = run_bass_kernel_spmd(nc, in_maps, core_ids=list(range(8)))
    return assemble_output(out.results)


if __name__ == "__main__":
    get_program()
    print("built ok")


# revision 19
# speedup vs baseline: 1.3286x; 1.1908x over previous
"""Trainium2 Bass kernel for nn_AttnAware (pixnorm->conv1x1 q/k attention + ResnetBlock).

Sharding: 8 cores = 4 batches x 2 query-halves. Each core receives its batch's
x [256, 4096] with pixel columns rotated so that its 2048 query pixels are the
first 2048 columns (attention is permutation-invariant over keys, and all
other ops are per-pixel). Single SPMD program, no collectives.

Per-core data layout: channels on partitions, pixels on free axis.
All PE matmuls run in 16/8-bit (1 cycle/row streaming and cheap LDWEIGHTS;
conv weights are converted to fp16 host-side and shipped in one packed DMA).
Attention works in the S^T orientation: S^T[j,i] tiles [128 keys, i-chunk]
computed as k_block^T @ q in fp16, exp on ACT (scale fused) writing fp8e4 P
pair-tiles, O^T accumulated with fp8 DoubleRow matmuls (two key blocks per
instruction, 0.5 cycles/row) against V^T pre-transposed per head on the PE.
The softmax denominator is accumulated on the PE as fp8 DoubleRow
ones-matmuls over the same P tiles, 1/D comes from the fast approximate DVE
reciprocal, is broadcast across partitions on the Pool engine, and O is
normalized inline per (head, i-pass). Pixelnorm row->tile broadcasts also go
through the Pool engine (partition_broadcast) instead of PE matmuls.
"""

import math
from contextlib import ExitStack

import numpy as np

import concourse.bass as bass
import concourse.mybir as mybir
import concourse.tile as tile
from concourse import bacc
from concourse.masks import make_identity

# ---------------- problem constants (hardcoded per contract) ----------------
B = 4
C = 256
HW = 64
N = HW * HW              # 4096 pixels
NQ = N // 2              # 2048 query pixels per core
NH = 2
HD = C // NH             # 128
CT = C // 128            # 2 channel tiles
C2T = 2 * C // 128       # 4 channel tiles for cat
JB = N // 128            # 32 key blocks
ATT_SCALE = HD ** -0.5
RATIO = 1.0 / (1.0 + 1e-8)   # PartialConv mask ratio (== 1.0f in fp32)
EPS = 1e-8
ISQ2 = 1.0 / math.sqrt(2.0)

# packed weight tile order (14 x [128, 256] fp16 column-tiles)
W_ORDER = [("wqT", CT), ("wkT", CT), ("wsT", C2T), ("w1T", C2T), ("w2T", CT)]
NWT = sum(n for _, n in W_ORDER)
# packed bias/alpha column order (18 x [128, 1] f32)
B_ORDER = [("bq", CT), ("bk", CT), ("b1", CT), ("bsc", CT), ("aq", CT),
           ("ak", CT), ("ar1", C2T), ("ar2", CT)]
NBC = sum(n for _, n in B_ORDER)

# ---------------- tuning knobs ----------------
IW = 1024                # i-columns per attention pass (PSUM S tile width)
LDW_OPT = False          # walrus ldw-opt is incompatible with 16/8-bit ldweights

f32 = mybir.dt.float32
f32r = mybir.dt.float32r
f16 = mybir.dt.float16
f8 = mybir.dt.float8e4
AF = mybir.ActivationFunctionType
OP = mybir.AluOpType
DR = mybir.MatmulPerfMode.DoubleRow


def build_program():
    nc = bacc.Bacc("TRN2", target_bir_lowering=False, debug=False)

    # register the pixnorm epsilon as a const AP usable as an ACT bias
    _eps_t = nc.alloc_sbuf_tensor(f"const-float32-{EPS}", [128, 1], f32)
    nc.gpsimd.memset(_eps_t.ap(), EPS)
    nc.const_aps.aps[(f32, EPS)] = _eps_t.ap()
    nc.all_engine_barrier()

    d = {}
    d["x"] = nc.dram_tensor("x", (C, N), f32, kind="ExternalInput").ap()
    d["wpack"] = nc.dram_tensor("wpack", (128, NWT * 256), f16,
                                kind="ExternalInput").ap()
    d["bpack"] = nc.dram_tensor("bpack", (128, NBC), f32,
                                kind="ExternalInput").ap()
    d["y"] = nc.dram_tensor("y", (C, NQ), f32, kind="ExternalOutput").ap()

    with tile.TileContext(nc) as tc:
        _body(tc, nc, d)
    nc.compile()
    return nc


def _body(tc, nc, d):
    x_d, y_d = d["x"], d["y"]

    with ExitStack() as top:
        const = top.enter_context(tc.tile_pool(name="const", bufs=1))
        wts = top.enter_context(tc.tile_pool(name="wts", bufs=1))

        ident16 = const.tile([128, 128], f16, tag="ident16", name="ident16")
        make_identity(nc, ident16[:])
        ones_col16 = const.tile([128, 1], f16, tag="ones_col16", name="ones_col16")
        nc.vector.memset(ones_col16[:], 1.0)
        ones_dr8 = const.tile([128, 2, 32], f8, tag="ones_dr8", name="ones_dr8")
        nc.vector.memset(ones_dr8[:], 1.0)

        # tensors that live into phase C
        with tc.tile_pool(name="oout", bufs=1) as oout:
            osb = [oout.tile([128, NQ], f16, tag=f"o{h}", name=f"o{h}")
                   for h in range(NH)]
            x16 = [oout.tile([128, N], f16, tag=f"x16_{h}", name=f"x16_{h}")
                   for h in range(NH)]

            # kqv: tensors that live from phase A through attention; closed
            # explicitly before the ResnetBlock pools open to reuse SBUF
            kqv_stack = ExitStack()
            kqv = kqv_stack.enter_context(tc.tile_pool(name="kqv", bufs=1))
            vt = [kqv.tile([128, N], f8, tag=f"vt{h}", name=f"vt{h}") for h in range(NH)]
            kt = [kqv.tile([128, N], f16, tag=f"k{h}", name=f"k{h}") for h in range(NH)]
            qt = [kqv.tile([128, NQ], f16, tag=f"q{h}", name=f"q{h}") for h in range(NH)]

            # x DMA first (critical path): 4 transfers, first pixel-half first
            front_stack = ExitStack()
            front = front_stack.enter_context(tc.tile_pool(name="front", bufs=1))
            xt = [front.tile([128, N], f32, tag=f"x{ct}", name=f"x{ct}")
                  for ct in range(CT)]
            # pixel half 0 on the SP ring, half 1 on the ACT ring (parallel)
            for ct in range(CT):
                nc.sync.dma_start(
                    xt[ct][:, :NQ], x_d[ct * 128:(ct + 1) * 128, :NQ])
            for ct in range(CT):
                nc.scalar.dma_start(
                    xt[ct][:, NQ:], x_d[ct * 128:(ct + 1) * 128, NQ:])

            # packed weights (one DMA) + packed biases/alphas (one DMA)
            wtile = wts.tile([128, NWT * 256], f16, tag="wpack", name="wpack")
            nc.sync.dma_start(wtile[:], d["wpack"])
            btile = wts.tile([128, NBC], f32, tag="bpack", name="bpack")
            nc.sync.dma_start(btile[:], d["bpack"])

            wv = {}
            off = 0
            for nm, n in W_ORDER:
                wv[nm] = [wtile[:, (off + i) * 256:(off + i + 1) * 256]
                          for i in range(n)]
                off += n
            bv = {}
            off = 0
            for nm, n in B_ORDER:
                bv[nm] = [btile[:, off + i:off + i + 1] for i in range(n)]
                off += n
            wqT, wkT, wsT, w1T, w2T = (wv[nm] for nm, _ in W_ORDER)
            bq, bk, b1, bsc, aq, ak, ar1, ar2 = (bv[nm] for nm, _ in B_ORDER)

            # =========== Phase A ===========
            with (
                tc.tile_pool(name="gtmp", bufs=6) as gtmp,
                tc.tile_pool(name="frow", bufs=2) as frow,
                tc.tile_pool(name="bcp", bufs=3) as bcp,
                tc.tile_pool(name="psA", bufs=2, space="PSUM") as psA,
                tc.tile_pool(name="psArow", bufs=2, space="PSUM") as psArow,
            ):
                # fp16 copy of x (DVE), per head tile
                for q in range(N // 1024):
                    for h in range(NH):
                        nc.vector.tensor_copy(
                            x16[h][:, q * 1024:(q + 1) * 1024],
                            xt[h][:, q * 1024:(q + 1) * 1024])

                # pixelnorm stats: ssum_c x^2 -> inv = exp(-0.5*ln(ssum/C+eps)),
                # per 512-column chunk; inv lands in an fp16 row, then Pool
                # broadcasts it across partitions.
                def inv_chunk(cc):
                    sqc = []
                    for ct in range(CT):
                        t = gtmp.tile([128, 512], f16, tag="g", name="sqch")
                        nc.gpsimd.tensor_tensor(
                            t[:], x16[ct][:, cc * 512:(cc + 1) * 512],
                            x16[ct][:, cc * 512:(cc + 1) * 512], op=OP.mult)
                        sqc.append(t)
                    ss = psArow.tile([1, 512], f32, tag="ssum", name="ssum")
                    for ct in range(CT):
                        nc.tensor.matmul(ss[:], ones_col16[:], sqc[ct][:],
                                         start=(ct == 0), stop=(ct == CT - 1))
                    lt = frow.tile([1, 512], f32, tag="lnt", name="lnt")
                    nc.scalar.activation(lt[:], ss[:], AF.Ln, bias=EPS, scale=1.0 / C)
                    iv = frow.tile([1, 512], f16, tag="inv", name="inv", bufs=8)
                    nc.scalar.activation(iv[:], lt[:], AF.Exp, scale=-0.5)
                    return iv

                # batch all pixelnorm stats first (single lnexp table residency)
                all_inv = [inv_chunk(cc) for cc in range(N // 512)]

                # per-chunk: bcast inv (Pool), xb = x*inv, gelu per conv, matmuls
                def conv_block(cc, convs):
                    bc = bcp.tile([128, 512], f16, tag="bc", name="bc")
                    nc.gpsimd.partition_broadcast(bc[:], all_inv[cc][:])
                    xb = []
                    for ct in range(CT):
                        t = gtmp.tile([128, 512], f16, tag="g", name="xb")
                        nc.vector.tensor_tensor(
                            t[:], x16[ct][:, cc * 512:(cc + 1) * 512], bc[:],
                            op=OP.mult)
                        xb.append(t)
                    for (wT, alpha, bias, out_tiles, oc0) in convs:
                        gchunks = []
                        for ct in range(CT):
                            g = gtmp.tile([128, 512], f16, tag="g", name="g")
                            nc.scalar.activation(g[:], xb[ct][:],
                                                 AF.Gelu, scale=alpha[ct])
                            gchunks.append(g)
                        osl = slice(oc0 + (cc % 4) * 512, oc0 + (cc % 4 + 1) * 512)
                        for mo in range(CT):
                            ps = psA.tile([128, 512], f32, tag="conv", name="conv")
                            for kc in range(CT):
                                nc.tensor.matmul(ps[:],
                                                 wT[kc][:, mo * 128:(mo + 1) * 128],
                                                 gchunks[kc][:],
                                                 start=(kc == 0), stop=(kc == CT - 1))
                            nc.vector.tensor_scalar(out_tiles[mo][:, osl],
                                                    ps[:], bias[mo], None, op0=OP.add)

                for cc in range(NQ // 512):
                    conv_block(cc, [(wqT, aq, bq, qt, 0), (wkT, ak, bk, kt, 0)])
                for cc in range(NQ // 512, N // 512):
                    conv_block(cc, [(wkT, ak, bk, kt, NQ)])

                # V^T per head: PE transpose of fp16 x, 4 blocks per PSUM slot,
                # copied out with a cast to fp8 (layout [(jb) d] flat == the
                # [jb/2, 2, d] DoubleRow view). Emitted after the convs so the
                # conv matmuls aren't stuck behind transposes that wait on the
                # second half of the x DMA; they fill the PE queue right before
                # attention needs vt.
                for h in range(NH):
                    for qb in range(JB // 4):
                        tp = psA.tile([128, 512], f16, tag="tp", name="tp")
                        for rr in range(4):
                            jb = qb * 4 + rr
                            nc.tensor.transpose(
                                tp[:, rr * 128:(rr + 1) * 128],
                                x16[h][:, jb * 128:(jb + 1) * 128], ident16[:])
                        nc.vector.tensor_copy(vt[h][:, qb * 512:(qb + 1) * 512], tp[:])

            front_stack.close()  # xt no longer needed (phase C uses x16)

            # =========== Phase B: attention (fp8 DoubleRow O and D) ==========
            with (
                tc.tile_pool(name="psS", bufs=2, space="PSUM") as psS,
                tc.tile_pool(name="psO", bufs=1, space="PSUM") as psO,
                tc.tile_pool(name="psD", bufs=1, space="PSUM") as psD,
                tc.tile_pool(name="pexp", bufs=3) as pexp,
                tc.tile_pool(name="drow", bufs=2) as drow_pool,
            ):
                NR = IW // 512
                NJP = JB // 2
                for h in range(NH):
                    for ip in range(NQ // IW):
                        i0 = ip * IW
                        o_ps = psO.tile([128, IW], f32, tag="o", name="o")
                        d_ps = psD.tile([32, IW], f32, tag="d", name="d")

                        def od_pair(jp, p2):
                            # O and D accumulation for pair jp (fp8 DoubleRow)
                            vt_dr = vt[h][:, jp * 256:(jp + 1) * 256].rearrange(
                                "p (j d) -> p j d", j=2)
                            for rr in range(NR):
                                psl = p2[:, :, rr * 512:(rr + 1) * 512]
                                nc.tensor.matmul(
                                    o_ps[:, rr * 512:(rr + 1) * 512],
                                    vt_dr, psl,
                                    start=(jp == 0), stop=(jp == NJP - 1),
                                    perf_mode=DR)
                                nc.tensor.matmul(
                                    d_ps[:, rr * 512:(rr + 1) * 512],
                                    ones_dr8[:], psl,
                                    start=(jp == 0), stop=(jp == NJP - 1),
                                    perf_mode=DR)

                        # software pipeline: emit O/D of pair jp-1 after the S
                        # matmuls of pair jp, so the in-order PE queue never
                        # heads into an O matmul whose exp isn't done yet
                        pending = None
                        for jp in range(NJP):
                            p2 = pexp.tile([128, 2, IW], f8, tag="p", name="p")
                            for jbi in range(2):
                                jb = jp * 2 + jbi
                                s_ps = psS.tile([128, IW], f32, tag="s", name="s")
                                for rr in range(NR):
                                    nc.tensor.matmul(
                                        s_ps[:, rr * 512:(rr + 1) * 512],
                                        kt[h][:, jb * 128:(jb + 1) * 128],
                                        qt[h][:, i0 + rr * 512:i0 + (rr + 1) * 512],
                                        start=True, stop=True)
                                nc.scalar.activation(p2[:, jbi, :], s_ps[:],
                                                     AF.Exp, scale=ATT_SCALE)
                            if pending is not None:
                                od_pair(*pending)
                            pending = (jp, p2)
                        od_pair(*pending)
                        dinv = drow_pool.tile([1, IW], f32, tag="dinv", name="dinv")
                        nc.vector.reciprocal_approx_fast(out=dinv[:], in_=d_ps[0:1, :])
                        # bcast 1/D across partitions (Pool) and normalize O
                        bc = drow_pool.tile([128, IW], f32, tag="bcD", name="bcD")
                        nc.gpsimd.partition_broadcast(bc[:], dinv[:])
                        nc.vector.tensor_tensor(
                            osb[h][:, i0:i0 + IW], o_ps[:], bc[:], op=OP.mult)

            # kqv pool (k/q/vt) closes here; back pool reuses its space
            kqv_stack.close()

            # ======= Phase C: ResnetBlock (per-512-column pipeline) =======
            with (
                tc.tile_pool(name="back", bufs=1) as back,
                tc.tile_pool(name="brow", bufs=4) as brow,
                tc.tile_pool(name="tmp", bufs=8) as tmp,
                tc.tile_pool(name="psB2", bufs=3, space="PSUM") as psB2,
                tc.tile_pool(name="psBrow2", bufs=2, space="PSUM") as psBrow2,
            ):
                NCC = NQ // 512
                cat = [osb[0], osb[1], x16[0], x16[1]]  # fp16, use [:, :NQ]

                def sl(t, cc):
                    return t[:, cc * 512:(cc + 1) * 512]

                def stats_chunk(tiles, nch, tag, cc):
                    # returns bcast [128,512] fp16 pixnorm scale for chunk cc
                    sq = []
                    for i, t in enumerate(tiles):
                        s = tmp.tile([128, 512], f16, tag="sq", name="sq", bufs=8)
                        nc.vector.tensor_tensor(s[:], sl(t, cc), sl(t, cc),
                                                op=OP.mult)
                        sq.append(s)
                    ss = psBrow2.tile([1, 512], f32, tag="ssum", name="ssum")
                    for i, s in enumerate(sq):
                        nc.tensor.matmul(ss[:], ones_col16[:], s[:],
                                         start=(i == 0), stop=(i == len(sq) - 1))
                    lt = brow.tile([1, 512], f32, tag="lnt", name="lnt")
                    nc.scalar.activation(lt[:], ss[:], AF.Ln, bias=EPS,
                                         scale=1.0 / nch)
                    iv = brow.tile([1, 512], f16, tag="iv", name="iv")
                    nc.scalar.activation(iv[:], lt[:], AF.Exp, scale=-0.5)
                    bc = tmp.tile([128, 512], f16, tag=f"bc{tag}",
                                  name=f"bc{tag}", bufs=4)
                    nc.gpsimd.partition_broadcast(bc[:], iv[:])
                    return bc

                # x_short convs first (independent of stats; keeps PE busy)
                xs = [back.tile([128, NQ], f32, tag=f"xs{mo}", name=f"xs{mo}")
                      for mo in range(CT)]
                for cc in range(NCC):
                    for mo in range(CT):
                        ps = psB2.tile([128, 512], f32, tag="conv", name="conv")
                        for kc in range(C2T):
                            nc.tensor.matmul(
                                ps[:], wsT[kc][:, mo * 128:(mo + 1) * 128],
                                sl(cat[kc], cc),
                                start=(kc == 0), stop=(kc == C2T - 1))
                        nc.vector.tensor_scalar(
                            sl(xs[mo], cc), ps[:],
                            RATIO * ISQ2, bsc[mo], op0=OP.mult, op1=OP.add)

                # r1 stats per chunk
                bc1 = [stats_chunk(cat, 2 * C, "r1", cc) for cc in range(NCC)]

                # gr1 = gelu(alpha_r1 * cat * invr1); h1 conv per chunk
                gr1 = [back.tile([128, NQ], f16, tag=f"gr1{ct}", name=f"gr1{ct}")
                       for ct in range(C2T)]
                h1 = [back.tile([128, NQ], f16, tag=f"h1{mo}", name=f"h1{mo}")
                      for mo in range(CT)]
                for cc in range(NCC):
                    for ct in range(C2T):
                        cn = tmp.tile([128, 512], f16, tag="cn", name="cn", bufs=6)
                        nc.vector.tensor_tensor(cn[:], sl(cat[ct], cc), bc1[cc][:],
                                                op=OP.mult)
                        nc.scalar.activation(sl(gr1[ct], cc), cn[:], AF.Gelu,
                                             scale=ar1[ct])
                    for mo in range(CT):
                        ps = psB2.tile([128, 512], f32, tag="conv", name="conv")
                        for kc in range(C2T):
                            nc.tensor.matmul(
                                ps[:], w1T[kc][:, mo * 128:(mo + 1) * 128],
                                sl(gr1[kc], cc),
                                start=(kc == 0), stop=(kc == C2T - 1))
                        nc.vector.tensor_scalar(
                            sl(h1[mo], cc), ps[:],
                            RATIO, b1[mo], op0=OP.mult, op1=OP.add)

                # r2 stats + gr2 + y per chunk
                bc2 = [stats_chunk(h1, C, "r2", cc) for cc in range(NCC)]
                gr2 = [back.tile([128, NQ], f16, tag=f"gr2{ct}", name=f"gr2{ct}")
                       for ct in range(CT)]
                yt = [back.tile([128, NQ], f32, tag=f"yt{mo}", name=f"yt{mo}")
                      for mo in range(CT)]
                for cc in range(NCC):
                    for ct in range(CT):
                        hn = tmp.tile([128, 512], f16, tag="cn", name="hn", bufs=6)
                        nc.vector.tensor_tensor(hn[:], sl(h1[ct], cc), bc2[cc][:],
                                                op=OP.mult)
                        nc.scalar.activation(sl(gr2[ct], cc), hn[:], AF.Gelu,
                                             scale=ar2[ct])
                    for mo in range(CT):
                        ps = psB2.tile([128, 512], f32, tag="conv", name="conv")
                        for kc in range(CT):
                            nc.tensor.matmul(
                                ps[:], w2T[kc][:, mo * 128:(mo + 1) * 128],
                                sl(gr2[kc], cc),
                                start=(kc == 0), stop=(kc == CT - 1))
                        nc.vector.scalar_tensor_tensor(
                            sl(yt[mo], cc), ps[:], RATIO * ISQ2,
                            sl(xs[mo], cc), op0=OP.mult, op1=OP.add)
                    if cc % 2 == 1:
                        for mo in range(CT):
                            nc.sync.dma_start(
                                y_d[mo * 128:(mo + 1) * 128,
                                    (cc - 1) * 512:(cc + 1) * 512],
                                yt[mo][:, (cc - 1) * 512:(cc + 1) * 512])


_PROGRAM = None


def get_program():
    global _PROGRAM
    if _PROGRAM is None:
        _PROGRAM = build_program()
    return _PROGRAM


def make_in_maps(inputs):
    x = np.asarray(inputs["x"], np.float32).reshape(B, C, N)
    col = lambda v, n: np.asarray(v, np.float32).reshape(n, 1)
    tr16 = lambda w: np.ascontiguousarray(
        np.asarray(w, np.float32).T).astype(np.float16)
    wmats = {"wqT": tr16(inputs["Wq"]), "wkT": tr16(inputs["Wk"]),
             "wsT": tr16(inputs["Ws"]), "w1T": tr16(inputs["W1"]),
             "w2T": tr16(inputs["W2"])}
    wpack = np.concatenate(
        [wmats[nm][i * 128:(i + 1) * 128, :]
         for nm, n in W_ORDER for i in range(n)], axis=1)
    bcols = {"bq": col(inputs["bq"], C), "bk": col(inputs["bk"], C),
             "b1": col(inputs["b1"], C),
             "bsc": ((col(inputs["bs"], C).astype(np.float64) +
                      col(inputs["b2"], C).astype(np.float64)) * ISQ2
                     ).astype(np.float32),
             "aq": col(inputs["alpha_q"], C), "ak": col(inputs["alpha_k"], C),
             "ar1": col(inputs["alpha_r1"], 2 * C),
             "ar2": col(inputs["alpha_r2"], C)}
    bpack = np.concatenate(
        [bcols[nm][i * 128:(i + 1) * 128, :]
         for nm, n in B_ORDER for i in range(n)], axis=1)
    shared = {"wpack": np.ascontiguousarray(wpack),
              "bpack": np.ascontiguousarray(bpack.astype(np.float32))}
    in_maps = []
    for b in range(B):
        for half in range(2):
            xp = (np.ascontiguousarray(x[b]) if half == 0
                  else np.ascontiguousarray(np.roll(x[b], -NQ, axis=1)))
            in_maps.append({"x": xp, **shared})
    return in_maps


def assemble_output(results):
    y = np.empty((B, C, N), np.float32)
    for core, res in enumerate(results):
        b, half = core // 2, core % 2
        y[b][:, half * NQ:(half + 1) * NQ] = res["y"]
    return y.reshape(B, C, HW, HW)


def _patch_ldw_opt():
    from concourse import bass_utils
    if getattr(bass_utils, "_ldw_patched", False):
        return
    orig = bass_utils.run_command

    def patched(argv, **kw):
        argv = ["--enable-ldw-opt=true" if a == "--enable-ldw-opt=false" else a
                for a in argv]
        return orig(argv, **kw)

    bass_utils.run_command = patched
    bass_utils._ldw_patched = True


def kernel(**inputs):
    from concourse.bass_utils import run_bass_kernel_spmd

    if LDW_OPT:
        _patch_ldw_opt()
    nc = get_program()
    in_maps = make_in_maps(inputs)
    out = run_bass_kernel_spmd(nc, in_maps, core_ids=list(range(8)))
    return assemble_output(out.results)


if __name__ == "__main__":
    get_program()
    print("built ok")
